# revision 31
# baseline (speedup 1.0000x reference)
"""Bottleneck-MHSA fused kernel for 8 Trainium2 NeuronCores (v2).

Sharding: core c = 2*b + s handles batch b; attention queries are split in
half between the two cores of a pair. Each core computes conv1 + BN1 + qkv
for its whole batch (redundantly with its pair partner), then attention for
all 4 heads over its query half, then BN2 + W3 + BN3 + residual for its
query half. Cross-core traffic is only three tiny BN-statistics AllReduces.

v2 changes vs v1 (531us):
- conv1 runs in bf16 (X + W1 host-cast) halving the X DMA that bounded
  phase 1; the residual reuses the resident bf16 X tiles (no re-load).
- BN sum/sumsq stats ride ScalarE activation accum_out (Copy / Square)
  instead of DVE reduce chains; BN applies are single fused
  Relu(scale,bias) activations split across Scalar/DVE/GpSimd.
- Softmax exp is split per query-group: ScalarE does real Exp on ~37%,
  GpSimd+DVE compute Schraudolph exp (one tensor_scalar into an int32
  bitcast) on the rest.  rel-err budget measured on CPU: ~8e-3 << 2e-2.
- Attention post-processing (1/denom broadcast-mult + BN2 stats) moved to
  GpSimd/DVE with double-buffered PSUM accumulators so head/group
  transitions no longer stall the PE (HAM stays warm).
- AllGather+local-reduce -> AllReduce; sqrt via DVE Newton rsqrt so the
  ScalarE activation table never leaves Exp.
- Dummy accumulate-matmul chains keep the PE clock at 2.4GHz through the
  BN1/BN2 collective windows.
"""
import numpy as np
import ml_dtypes

HEADS = 4
DH = 64
C = 256          # PLANES
CIN = 1024       # IN_PLANES
N = 2744         # tokens per batch
NQ = N // 2      # query half per core
B = 4
EPS = 1e-5
N_CORES = 8
CNT12 = 8 * N    # BN1 effective count (pairs double-count; mean/var exact)
CNT2 = 8 * NQ    # BN2/BN3 count (distinct shards)

SCHR_A = 12102203.161561485   # 2^23 / ln 2
SCHR_B = 1064986823.0         # 127*2^23 - 60801*8 (Schraudolph bias)


def _chunks(total, n):
    # even sizes (fp32r matmul requires an even moving free dim)
    assert total % 2 == 0
    half = total // 2
    sizes = [2 * (half // n + (1 if i < half % n else 0)) for i in range(n)]
    out, off = [], 0
    for s in sizes:
        out.append((off, s))
        off += s
    return out


# conv col groups (bf16 X tiles); first 3 cover the residual half 0:1372
CGX = [(0, 512), (512, 512), (1024, 348), (1372, 512), (1884, 512), (2396, 348)]
CH6 = _chunks(N, 6)     # key/token chunks for qkv (456/458 wide)
CH3 = _chunks(NQ, 3)    # query chunks for qkv / W3
MT22 = [(t * 128, min(128, N - t * 128)) for t in range((N + 127) // 128)]
WARM_K = 44             # dummy matmuls per keep-warm chain (~9.5us)

# X DMA groups: (col offset, width, conv sub-splits); first two resident
XGRP = [(0, 1024, [(0, 512), (512, 512)]), (1024, 348, [(0, 348)]),
        (1372, 1024, [(0, 512), (512, 512)]), (2396, 348, [(0, 348)])]
# tiled DRAM offsets: X is staged host-side as contiguous [128, w] blocks
XOFF = {}
_o = 0
for _g, (_off, _w, _) in enumerate(XGRP):
    for _k in range(8):
        XOFF[(_g, _k)] = _o
        _o += 128 * _w
OOFF = {}
_o = 0
for _mt in range(8):
    for _cg in range(3):
        OOFF[(_mt, _cg)] = _o
        _o += 128 * CGX[_cg][1]


def build_program():
    from concourse import bacc, mybir, tile

    F32 = mybir.dt.float32
    F32R = mybir.dt.float32r
    BF16 = mybir.dt.bfloat16
    I32 = mybir.dt.int32

    nc = bacc.Bacc("TRN2", target_bir_lowering=False, debug=False,
                   num_devices=N_CORES)

    # ---- I/O ----
    io = {}
    io["X"] = nc.dram_tensor("X", [CIN * N], BF16, kind="ExternalInput").ap()
    io["W1T"] = nc.dram_tensor("W1T", [CIN, C], BF16, kind="ExternalInput").ap()
    io["WQT"] = nc.dram_tensor("WQT", [C, C], F32R, kind="ExternalInput").ap()
    io["WVT"] = nc.dram_tensor("WVT", [C, C], F32R, kind="ExternalInput").ap()
    io["W3T"] = nc.dram_tensor("W3T", [C, CIN], F32R, kind="ExternalInput").ap()
    io["WKQ"] = nc.dram_tensor("WKQ", [HEADS, C, 128], F32R, kind="ExternalInput").ap()
    io["REL"] = nc.dram_tensor("REL", [HEADS, DH, NQ], F32R, kind="ExternalInput").ap()
    io["BKQ"] = nc.dram_tensor("BKQ", [HEADS, 128], F32, kind="ExternalInput").ap()
    io["BQ"] = nc.dram_tensor("BQ", [HEADS, DH], F32, kind="ExternalInput").ap()
    io["BVR"] = nc.dram_tensor("BVR", [1, C], F32R, kind="ExternalInput").ap()
    io["GB1"] = nc.dram_tensor("GB1", [2, C], F32, kind="ExternalInput").ap()
    io["GB2"] = nc.dram_tensor("GB2", [2, C], F32, kind="ExternalInput").ap()
    io["GB3"] = nc.dram_tensor("GB3", [2, CIN], F32, kind="ExternalInput").ap()
    io["OUT"] = nc.dram_tensor("OUT", [CIN * NQ], F32, kind="ExternalOutput").ap()

    with tile.TileContext(nc) as tc:
        _emit(nc, tc, mybir, F32, F32R, BF16, I32, io)

    nc.compile()
    from concourse.bass_interp import get_hw_module
    nc.m = get_hw_module(nc.m)
    return nc


def _emit(nc, tc, mybir, F32, F32R, BF16, I32, io):
    import contextlib

    AX = mybir.AluOpType
    AF = mybir.ActivationFunctionType
    X_AXIS = mybir.AxisListType.X

    Xd, W1T, WQT, WVT, W3T = io["X"], io["W1T"], io["WQT"], io["WVT"], io["W3T"]
    WKQ, RELd, BKQ, BQd, BVR = io["WKQ"], io["REL"], io["BKQ"], io["BQ"], io["BVR"]
    GB1, GB2, GB3, OUTd = io["GB1"], io["GB2"], io["GB3"], io["OUT"]

    def stats_allreduce(src_sbuf, width, out_sbuf, tag):
        """Sum [P, width] stats over all 8 cores into out_sbuf (AllGather +
        local reduce; measured 2x faster than the AllReduce collective for
        these tiny payloads)."""
        p = src_sbuf.shape[0]
        cin = dpool.tile([p, width], F32, name=f"arin_{tag}")
        cout = dpool.tile([N_CORES, p, width], F32, addr_space="Shared",
                          name=f"arout_{tag}")
        nc.sync.dma_start(cin[:], src_sbuf[:])
        nc.gpsimd.collective_compute(
            "AllGather", AX.bypass,
            replica_groups=[list(range(N_CORES))],
            ins=[cin.opt()], outs=[cout.opt()],
        )
        gath = wpool.tile([p, width, N_CORES], F32, name=f"gath_{tag}")
        nc.sync.dma_start(gath[:], cout[:].rearrange("g p c -> p c g"))
        nc.vector.reduce_sum(out_sbuf[:], gath[:], X_AXIS)

    def rsqrt_newton(y, x, tag):
        """y = 1/sqrt(x) on DVE only (bit-trick seed + 2 Newton steps)."""
        p, w = x.shape[0], x.shape[1]
        xi = x[:].bitcast(I32)
        t1 = wpool.tile([p, w], I32, name=f"rsq_t1_{tag}")
        nc.vector.tensor_scalar(t1[:], xi, 1, None, AX.arith_shift_right)
        yi = y[:].bitcast(I32)
        nc.vector.tensor_scalar(yi, t1[:], -1, 0x5f3759df, AX.mult, AX.add)
        h = wpool.tile([p, w], F32, name=f"rsq_h_{tag}")
        for _ in range(2):
            nc.vector.tensor_tensor(h[:], y[:], y[:], AX.mult)
            nc.vector.tensor_tensor(h[:], x[:], h[:], AX.mult)
            nc.vector.tensor_scalar(h[:], h[:], -0.5, 1.5, AX.mult, AX.add)
            nc.vector.tensor_tensor(y[:], y[:], h[:], AX.mult)

    def bn_coeffs(tot, gt, bt, cnt, w, sc, cc, tag):
        """tot [P, 2w] = [sums | sumsqs] -> scale sc [P, w], bias cc [P, w].
        All DVE (Newton rsqrt) so ScalarE keeps its Exp table loaded."""
        p = tot.shape[0]
        mean = wpool.tile([p, w], F32, name=f"mean_{tag}")
        var = wpool.tile([p, w], F32, name=f"var_{tag}")
        nc.vector.tensor_scalar_mul(mean[:], tot[:, 0:w], 1.0 / cnt)
        nc.vector.tensor_scalar_mul(var[:], tot[:, w:2 * w], 1.0 / cnt)
        m2 = wpool.tile([p, w], F32, name=f"m2_{tag}")
        nc.vector.tensor_tensor(m2[:], mean[:], mean[:], AX.mult)
        nc.vector.tensor_tensor(var[:], var[:], m2[:], AX.subtract)
        nc.vector.tensor_scalar_add(var[:], var[:], EPS)
        rstd = wpool.tile([p, w], F32, name=f"rstd_{tag}")
        rsqrt_newton(rstd, var, tag)
        nc.vector.tensor_tensor(sc[:], gt[:], rstd[:], AX.mult)
        tmp = wpool.tile([p, w], F32, name=f"tmpc_{tag}")
        nc.vector.tensor_tensor(tmp[:], sc[:], mean[:], AX.mult)
        nc.vector.tensor_tensor(cc[:], bt[:], tmp[:], AX.subtract)

    with contextlib.ExitStack() as top:
        wpool = top.enter_context(tc.tile_pool(name="wpool", bufs=1))
        dpool = top.enter_context(tc.tile_pool(name="dpool", bufs=1, space="DRAM"))

        # ---- weights / constants ----
        w1t = []
        for k in range(8):
            t = wpool.tile([128, C], BF16, name=f"w1t{k}")
            nc.scalar.dma_start(t[:], W1T[k * 128:(k + 1) * 128, :])
            w1t.append(t)
        wqt, wvt = [], []
        for srcw, dst, nm in ((WQT, wqt, "wq"), (WVT, wvt, "wv")):
            for k in range(2):
                t = wpool.tile([128, C], F32R, name=f"{nm}{k}")
                nc.scalar.dma_start(t[:], srcw[k * 128:(k + 1) * 128, :])
                dst.append(t)
        wkqt = []
        for h in range(HEADS):
            row = []
            for k in range(2):
                t = wpool.tile([128, 128], F32R, name=f"wkq{h}_{k}")
                nc.scalar.dma_start(t[:], WKQ[h][k * 128:(k + 1) * 128, :])
                row.append(t)
            wkqt.append(row)
        w3t = []
        for k in range(2):
            t = wpool.tile([128, CIN], F32R, name=f"w3t{k}")
            nc.scalar.dma_start(t[:], W3T[k * 128:(k + 1) * 128, :])
            w3t.append(t)
        bvrow = wpool.tile([1, C], F32R, name="bvrow")
        nc.scalar.dma_start(bvrow[:], BVR[:])

        bkqt = wpool.tile([128, HEADS], F32, name="bkqt")
        nc.scalar.dma_start(bkqt[:], BKQ[:].rearrange("h p -> p h"))
        bqt = wpool.tile([DH, HEADS], F32, name="bqt")
        nc.scalar.dma_start(bqt[:], BQd[:].rearrange("h d -> d h"))
        g1t = wpool.tile([128, 2], F32, name="g1t")
        b1t = wpool.tile([128, 2], F32, name="b1t")
        nc.scalar.dma_start(g1t[:], GB1[0].rearrange("(m p) -> p m", p=128))
        nc.scalar.dma_start(b1t[:], GB1[1].rearrange("(m p) -> p m", p=128))
        g2t = wpool.tile([DH, HEADS], F32, name="g2t")
        b2t = wpool.tile([DH, HEADS], F32, name="b2t")
        nc.scalar.dma_start(g2t[:], GB2[0].rearrange("(h d) -> d h", d=DH))
        nc.scalar.dma_start(b2t[:], GB2[1].rearrange("(h d) -> d h", d=DH))
        g3t = wpool.tile([128, 8], F32, name="g3t")
        b3t = wpool.tile([128, 8], F32, name="b3t")
        nc.scalar.dma_start(g3t[:], GB3[0].rearrange("(m p) -> p m", p=128))
        nc.scalar.dma_start(b3t[:], GB3[1].rearrange("(m p) -> p m", p=128))

        # constants: ones pad (bitcast to f32r where needed), warm-chain srcs
        onespad = wpool.tile([128, 128], F32, name="onespad")
        nc.vector.memset(onespad[:], 1.0)
        warmx = wpool.tile([128, 512], F32, name="warmx")
        nc.vector.memset(warmx[:], 0.0)

        def warm_chain(pool, tag, shape, bufs=1, k=WARM_K):
            """Dummy accumulate-matmul chain: keeps the PE clock warm during
            a collective window.  Allocates from the given live PSUM pool."""
            wps = pool.tile(shape, F32, name=f"warm_{tag}", tag=tag, bufs=bufs)
            for i in range(k):
                nc.tensor.matmul(wps[0:128, 0:512], onespad[:].bitcast(F32R),
                                 warmx[:].bitcast(F32R),
                                 start=(i == 0), stop=(i == k - 1))

        # stats accumulators
        S1 = wpool.tile([128, 12], F32, name="S1")   # conv1 sums   (mt*6+cg)
        Q1 = wpool.tile([128, 12], F32, name="Q1")   # conv1 sumsqs
        S2 = wpool.tile([DH, 12], F32, name="S2")    # attn sums    (h*3+gi)
        Q2 = wpool.tile([DH, 12], F32, name="Q2")
        S3 = wpool.tile([128, 24], F32, name="S3")   # W3 sums      (mt*3+ci)
        Q3 = wpool.tile([128, 24], F32, name="Q3")

        # resident bf16 X tiles covering cols 0:1372 (conv input + residual)
        XAB = [wpool.tile([128, 1024], BF16, name=f"xab{k}") for k in range(8)]
        XC = [wpool.tile([128, 348], BF16, name=f"xc{k}") for k in range(8)]
        # residual slices per finale chunk cg in 0..2
        XRES = [[XAB[k][:, 0:512] for k in range(8)],
                [XAB[k][:, 512:1024] for k in range(8)],
                [XC[k][:] for k in range(8)]]
        OUT2 = [wpool.tile([128, NQ], F32R, name=f"out2_{m}") for m in range(2)]

        with contextlib.ExitStack() as ph_a:
            qpool = ph_a.enter_context(tc.tile_pool(name="qpool", bufs=1))
            KHAT = [qpool.tile([128, N], F32R, name=f"khat{h}") for h in range(HEADS)]
            QHAT = [qpool.tile([128, NQ], F32R, name=f"qhat{h}") for h in range(HEADS)]
            # per-head blocks of 22*65 cols: [v^T (64) | ones] per token tile
            VTON = qpool.tile([128, HEADS * 22 * 65], F32R, name="vton")
            # ones columns (softmax denominator rows), one strided copy per head
            for h in range(HEADS):
                dst = VTON[:, h * 1430:(h + 1) * 1430].rearrange(
                    "p (t c) -> p t c", t=22)[:, :, DH:DH + 1]
                src = onespad[:, 0:22].rearrange("p (t c) -> p t c", t=22)
                nc.vector.tensor_copy(dst, src)
            for h in range(HEADS):
                nc.scalar.dma_start(QHAT[h][DH:128, :], RELd[h])

            with contextlib.ExitStack() as ph1:
                y1pool = ph1.enter_context(tc.tile_pool(name="y1pool", bufs=1))
                Y1 = [y1pool.tile([128, N], F32R, name=f"y1_{m}") for m in range(2)]
                sq1pool = ph1.enter_context(tc.tile_pool(name="sq1pool", bufs=2))

                # ---- phase 1: conv1 (y1 = W1 @ x) bf16, stats on ScalarE ----
                with tc.tile_pool(name="xbpool", bufs=2) as xbpool, \
                     tc.tile_pool(name="psum1", bufs=4, space="PSUM") as psum1:
                    # warm the PE clock while weights/X stream in
                    warm_chain(psum1, "warm0", [128, 512], k=16)
                    dma_engs = (nc.sync, nc.gpsimd, nc.scalar)
                    cstat = 0
                    for g, (gbase, gw, subs) in enumerate(XGRP):
                        if g == 0:
                            xts = XAB
                        elif g == 1:
                            xts = XC
                        else:
                            xts = [xbpool.tile([128, gw], BF16,
                                               name=f"xb{g}_{k}", tag=f"xb{k}")
                                   for k in range(8)]
                        for k in range(8):
                            xo = XOFF[(g, k)]
                            dma_engs[k % 3].dma_start(
                                xts[k][:], Xd[xo:xo + 128 * gw].rearrange(
                                    "(p f) -> p f", p=128))
                        for so, ssz in subs:
                            off = gbase + so
                            for mt in range(2):
                                ps = psum1.tile([128, ssz], F32, name="pconv",
                                                tag="pconv",
                                                padded_shape=[128, 512])
                                for k in range(8):
                                    nc.tensor.matmul(
                                        ps[:], w1t[k][:, mt * 128:(mt + 1) * 128],
                                        xts[k][:, so:so + ssz],
                                        start=(k == 0), stop=(k == 7))
                                idx = mt * 6 + cstat
                                nc.scalar.activation(Y1[mt][:, off:off + ssz],
                                                     ps[:], AF.Copy,
                                                     accum_out=S1[:, idx:idx + 1])
                                sq = sq1pool.tile([128, ssz], F32, name="sqs",
                                                  tag="sqs",
                                                  padded_shape=[128, 512])
                                nc.scalar.activation(sq[:], ps[:], AF.Square,
                                                     accum_out=Q1[:, idx:idx + 1])
                            cstat += 1

                # ---- phase 1b: BN1 AllReduce + coeffs (DVE); PE keeps warm ----
                s1sum = wpool.tile([128, 2], F32, name="s1sum")
                q1sum = wpool.tile([128, 2], F32, name="q1sum")
                nc.vector.reduce_sum(s1sum[:], S1[:].rearrange("p (m c) -> p m c", c=6), X_AXIS)
                nc.vector.reduce_sum(q1sum[:], Q1[:].rearrange("p (m c) -> p m c", c=6), X_AXIS)
                st1 = wpool.tile([128, 4], F32, name="st1")
                nc.vector.tensor_copy(st1[:, 0:2], s1sum[:])
                nc.vector.tensor_copy(st1[:, 2:4], q1sum[:])
                tot1 = wpool.tile([128, 4], F32, name="tot1")
                stats_allreduce(st1, 4, tot1, "bn1")
                s1c = wpool.tile([128, 2], F32, name="s1c")
                c1c = wpool.tile([128, 2], F32, name="c1c")
                bn_coeffs(tot1, g1t, b1t, CNT12, 2, s1c, c1c, "bn1")

                # ---- phase 2: out1 = relu(s*y1+c) in place, 3-engine split ----
                OUT1 = [Y1[m][:] for m in range(2)]
                for mt in range(2):
                    for cg, (off, sz) in enumerate(CGX):
                        ap = Y1[mt][:, off:off + sz]
                        if cg < 4:
                            nc.scalar.activation(ap, ap, AF.Relu,
                                                 bias=c1c[:, mt:mt + 1],
                                                 scale=s1c[:, mt:mt + 1])
                        else:
                            nc.vector.tensor_scalar(ap, ap,
                                                    s1c[:, mt:mt + 1],
                                                    c1c[:, mt:mt + 1],
                                                    AX.mult, AX.add)
                            nc.vector.tensor_scalar(ap, ap, 0.0, None, AX.max)

                # ---- phase 3a: vT = out1^T @ WvT (+bv), into VTON (ScalarE) ----
                with tc.tile_pool(name="psum3", bufs=1, space="PSUM") as psum3:
                    # keep the PE clock warm through the BN1 collective window
                    warm_chain(psum3, "warm1", [128, 512])
                    for t, (mo, msz) in enumerate(MT22):
                        ps = psum3.tile([128, C], F32, name="pvt", tag="pvt", bufs=2)
                        nc.tensor.matmul(ps[0:msz, :],
                                         onespad[0:1, 0:msz].bitcast(F32R),
                                         bvrow[:], start=True, stop=False)
                        for k in range(2):
                            nc.tensor.matmul(ps[0:msz, :], OUT1[k][:, mo:mo + msz],
                                             wvt[k][:], start=False, stop=(k == 1))
                        # one strided copy for all 4 heads: [msz, 4, 64]
                        dst = VTON[0:msz, :].rearrange(
                            "p (h c) -> p h c", h=HEADS)[:, :, t * 65:t * 65 + DH]
                        src = ps[0:msz, :].rearrange("p (h c) -> p h c", h=HEADS)
                        nc.scalar.activation(dst, src, AF.Copy)

                    # ---- phase 3b: KHAT = [k;q] packed, QHAT q-half ----
                    # bias adds split Scalar (Identity+bias) / DVE; GpSimd
                    # cannot read PSUM.
                    for h in range(HEADS):
                        hs = h * DH
                        for ci, (off, sz) in enumerate(CH6):
                            ps = psum3.tile([128, sz], F32, name="pkh", tag="pkh",
                                            bufs=3)
                            for k in range(2):
                                nc.tensor.matmul(ps[:], wkqt[h][k][:],
                                                 OUT1[k][:, off:off + sz],
                                                 start=(k == 0), stop=(k == 1))
                            if ci % 2 == 0:
                                nc.scalar.activation(KHAT[h][:, off:off + sz],
                                                     ps[:], AF.Identity,
                                                     bias=bkqt[:, h:h + 1])
                            else:
                                nc.vector.tensor_scalar(KHAT[h][:, off:off + sz],
                                                        ps[:], bkqt[:, h:h + 1],
                                                        None, AX.add)
                        for (off, sz) in CH3:
                            pq = psum3.tile([DH, sz], F32, name="pqh", tag="pqh",
                                            bufs=2)
                            for k in range(2):
                                nc.tensor.matmul(pq[:], wqt[k][:, hs:hs + DH],
                                                 OUT1[k][:, off:off + sz],
                                                 start=(k == 0), stop=(k == 1))
                            nc.vector.tensor_scalar(QHAT[h][0:DH, off:off + sz], pq[:],
                                                    bqt[:, h:h + 1], None, AX.add)

            # ---- phase 4: attention ----
            # queries 0:1024 per head run as a Scalar-dense 1024-wide loop;
            # the 348-query tails of two heads are interleaved afterwards so
            # their serial logits->exp->AV chains overlap.  The softmax
            # denominator reciprocal is exp(-ln(d)) on ScalarE (a [1,512]
            # DVE reciprocal costs 3us; ScalarE does the pair in 1.1us).
            with tc.tile_pool(name="oattp", bufs=1) as oattp, \
                 tc.tile_pool(name="epool", bufs=1) as epool:
                OATT = [oattp.tile([DH, NQ], F32R, name=f"oatt{h}")
                        for h in range(HEADS)]

                def post_group(pool, pav, h, off, sz, idx, pbtag, pbbufs):
                    dln = epool.tile([1, sz], F32, name="dln", tag="dln",
                                     bufs=2, padded_shape=[1, 512])
                    nc.scalar.activation(dln[:], pav[DH:65, :], AF.Ln)
                    rcr = epool.tile([1, sz], F32, name="rcr", tag="rcr",
                                     bufs=2, padded_shape=[1, 512])
                    nc.scalar.activation(rcr[:], dln[:], AF.Exp, scale=-1.0)
                    pb = pool.tile([DH, sz], F32, name="pb", tag=pbtag,
                                   bufs=pbbufs, padded_shape=[DH, 512])
                    nc.tensor.matmul(pb[:], onespad[0:1, 0:DH], rcr[:],
                                     start=True, stop=True)
                    pbs = epool.tile([DH, sz], F32, name="pbs", tag="pbs",
                                     bufs=2, padded_shape=[DH, 512])
                    nc.vector.tensor_copy(pbs[:], pb[:])
                    nc.vector.scalar_tensor_tensor(
                        OATT[h][:, off:off + sz], pav[0:DH, :], 1.0, pbs[:],
                        AX.mult, AX.mult, accum_out=S2[:, idx:idx + 1])
                    sq2 = epool.tile([DH, sz], F32, name="sq2", tag="sq2",
                                     bufs=2, padded_shape=[DH, 512])
                    nc.vector.scalar_tensor_tensor(
                        sq2[:], OATT[h][:, off:off + sz], 1.0,
                        OATT[h][:, off:off + sz],
                        AX.mult, AX.mult, accum_out=Q2[:, idx:idx + 1])

                with tc.tile_pool(name="psum4a", bufs=1, space="PSUM") as ps4a:
                    for h in range(HEADS):
                        pavs = [ps4a.tile([65, 512], F32, name=f"pav{si}",
                                          tag=f"pav{si}", bufs=1)
                                for si in range(2)]
                        for t, (mo, msz) in enumerate(MT22):
                            ps = ps4a.tile([128, 1024], F32, name="ps", tag="ps",
                                           bufs=2)
                            for so in (0, 512):
                                nc.tensor.matmul(ps[0:msz, so:so + 512],
                                                 KHAT[h][:, mo:mo + msz],
                                                 QHAT[h][:, so:so + 512],
                                                 start=True, stop=True)
                            e = epool.tile([128, 1024], F32R, name="e", tag="e",
                                           bufs=3)
                            nc.scalar.activation(e[0:msz, :], ps[0:msz, :],
                                                 AF.Exp)
                            base = (h * 22 + t) * 65
                            for si, so in enumerate((0, 512)):
                                nc.tensor.matmul(pavs[si][:],
                                                 VTON[0:msz, base:base + 65],
                                                 e[0:msz, so:so + 512],
                                                 start=(t == 0), stop=(t == 21))
                        for si in range(2):
                            post_group(ps4a, pavs[si], h, si * 512, 512,
                                       h * 3 + si, "pb", 1)

                with tc.tile_pool(name="psum4b", bufs=1, space="PSUM") as ps4b:
                    for pair in ((0, 1), (2, 3)):
                        pav2 = {h: ps4b.tile([65, 348], F32, name=f"pav2_{h}",
                                             tag=f"pav2_{h % 2}", bufs=1,
                                             padded_shape=[65, 512])
                                for h in pair}
                        for t, (mo, msz) in enumerate(MT22):
                            for h in pair:
                                ps = ps4b.tile([128, 348], F32, name="ps2",
                                               tag="ps2", bufs=4,
                                               padded_shape=[128, 512])
                                nc.tensor.matmul(ps[0:msz, :],
                                                 KHAT[h][:, mo:mo + msz],
                                                 QHAT[h][:, 1024:1372],
                                                 start=True, stop=True)
                                e2 = epool.tile([128, 348], F32R, name="e2",
                                                tag="e2", bufs=4,
                                                padded_shape=[128, 512])
                                nc.scalar.activation(e2[0:msz, :], ps[0:msz, :],
                                                     AF.Exp)
                                base = (h * 22 + t) * 65
                                nc.tensor.matmul(pav2[h][:],
                                                 VTON[0:msz, base:base + 65],
                                                 e2[0:msz, :],
                                                 start=(t == 0), stop=(t == 21))
                        for h in pair:
                            post_group(ps4b, pav2[h], h, 1024, 348,
                                       h * 3 + 2, "pb2", 2)
                            # move raw head output into OUT2 layout
                            nc.sync.dma_start(
                                OUT2[h // 2][(h % 2) * DH:(h % 2) * DH + DH, :],
                                OATT[h][:])

                # ---- phase 5: BN2 AllReduce + fused Relu apply ----
                s2sum = wpool.tile([DH, 4], F32, name="s2sum")
                q2sum = wpool.tile([DH, 4], F32, name="q2sum")
                nc.vector.reduce_sum(s2sum[:], S2[:].rearrange("p (h c) -> p h c", c=3), X_AXIS)
                nc.vector.reduce_sum(q2sum[:], Q2[:].rearrange("p (h c) -> p h c", c=3), X_AXIS)
                st2 = wpool.tile([DH, 8], F32, name="st2")
                nc.vector.tensor_copy(st2[:, 0:4], s2sum[:])
                nc.vector.tensor_copy(st2[:, 4:8], q2sum[:])
                tot2 = wpool.tile([DH, 8], F32, name="tot2")
                stats_allreduce(st2, 8, tot2, "bn2")
                s2c = wpool.tile([DH, 4], F32, name="s2c")
                c2c = wpool.tile([DH, 4], F32, name="c2c")
                bn_coeffs(tot2, g2t, b2t, CNT2, 4, s2c, c2c, "bn2")
                # rearrange [64,4] head coeffs -> [128,2] OUT2 channel layout
                s2c128 = wpool.tile([128, 2], F32, name="s2c128")
                c2c128 = wpool.tile([128, 2], F32, name="c2c128")
                for s in range(2):
                    nc.sync.dma_start(
                        s2c128[s * DH:(s + 1) * DH, :],
                        s2c[:].rearrange("d (m s) -> d m s", s=2)[:, :, s])
                    nc.sync.dma_start(
                        c2c128[s * DH:(s + 1) * DH, :],
                        c2c[:].rearrange("d (m s) -> d m s", s=2)[:, :, s])
                for m in range(2):
                    nc.scalar.activation(OUT2[m][:], OUT2[m][:], AF.Relu,
                                         bias=c2c128[:, m:m + 1],
                                         scale=s2c128[:, m:m + 1])

        # ---- phase 6: y3 = W3 @ out2; stats Scalar+DVE; finale 3-engine ----
        with tc.tile_pool(name="y3pool", bufs=1) as y3pool, \
             tc.tile_pool(name="fpool", bufs=2) as fpool, \
             tc.tile_pool(name="psum6", bufs=3, space="PSUM") as psum6:
            # keep the PE clock warm through the BN2 collective window
            warm_chain(psum6, "warm2", [128, 512], bufs=1)
            Y3 = [y3pool.tile([128, NQ], BF16, name=f"y3_{m}") for m in range(8)]
            for mt in range(8):
                for ci, (off, sz) in enumerate(CH3):
                    ps = psum6.tile([128, sz], F32, name="pw3", tag="pw3")
                    for k in range(2):
                        nc.tensor.matmul(ps[:], w3t[k][:, mt * 128:(mt + 1) * 128],
                                         OUT2[k][:, off:off + sz],
                                         start=(k == 0), stop=(k == 1))
                    idx = mt * 3 + ci
                    nc.scalar.activation(Y3[mt][:, off:off + sz], ps[:], AF.Copy,
                                         accum_out=S3[:, idx:idx + 1])
                    sq = fpool.tile([128, sz], F32, name="sq3", tag="sq3")
                    nc.vector.scalar_tensor_tensor(
                        sq[:], Y3[mt][:, off:off + sz], 1.0,
                        Y3[mt][:, off:off + sz], AX.mult, AX.mult,
                        accum_out=Q3[:, idx:idx + 1])

            s3sum = wpool.tile([128, 8], F32, name="s3sum")
            q3sum = wpool.tile([128, 8], F32, name="q3sum")
            nc.vector.reduce_sum(s3sum[:], S3[:].rearrange("p (m c) -> p m c", c=3), X_AXIS)
            nc.vector.reduce_sum(q3sum[:], Q3[:].rearrange("p (m c) -> p m c", c=3), X_AXIS)
            st3 = wpool.tile([128, 16], F32, name="st3")
            nc.vector.tensor_copy(st3[:, 0:8], s3sum[:])
            nc.vector.tensor_copy(st3[:, 8:16], q3sum[:])
            tot3 = wpool.tile([128, 16], F32, name="tot3")
            stats_allreduce(st3, 16, tot3, "bn3")
            s3c = wpool.tile([128, 8], F32, name="s3c")
            c3c = wpool.tile([128, 8], F32, name="c3c")
            bn_coeffs(tot3, g3t, b3t, CNT2, 8, s3c, c3c, "bn3")

            # finale: out = relu(s3*y3 + c3 + x); op1 on DVE (stt with the
            # resident bf16 X tiles), op2 split Scalar (cg 0,2) / DVE (cg 1)
            for mt in range(8):
                for cg in range(3):
                    off, sz = CGX[cg]
                    tf = fpool.tile([128, sz], F32, name="tf", tag="tf", bufs=3,
                                    padded_shape=[128, 512])
                    nc.vector.scalar_tensor_tensor(
                        tf[:], Y3[mt][:, off:off + sz], s3c[:, mt:mt + 1],
                        XRES[cg][mt], AX.mult, AX.add)
                    to = fpool.tile([128, sz], F32, name="to", tag="to", bufs=3,
                                    padded_shape=[128, 512])
                    if cg != 1:
                        nc.scalar.activation(to[:], tf[:], AF.Relu,
                                             bias=c3c[:, mt:mt + 1])
                    else:
                        nc.vector.tensor_scalar(to[:], tf[:], c3c[:, mt:mt + 1],
                                                0.0, AX.add, AX.max)
                    oo = OOFF[(mt, cg)]
                    nc.sync.dma_start(OUTd[oo:oo + 128 * sz].rearrange(
                        "(p f) -> p f", p=128), to[:])


_NC_CACHE = {}


def _get_program():
    if "nc" not in _NC_CACHE:
        _NC_CACHE["nc"] = build_program()
    return _NC_CACHE["nc"]


def _host_prep(inputs):
    x = np.ascontiguousarray(inputs["x"].reshape(B, CIN, N))
    rel = (inputs["rel_h"] + inputs["rel_w"] + inputs["rel_d"]).reshape(HEADS, DH, N)
    rel = np.ascontiguousarray(rel.astype(np.float32))
    W1T = np.ascontiguousarray(inputs["W1"].T.astype(ml_dtypes.bfloat16))
    WQT = np.ascontiguousarray(inputs["Wq"].T.astype(np.float32))
    WKT = np.ascontiguousarray(inputs["Wk"].T.astype(np.float32))
    WVT = np.ascontiguousarray(inputs["Wv"].T.astype(np.float32))
    W3T = np.ascontiguousarray(inputs["W3"].T.astype(np.float32))
    WKQ = np.stack([np.concatenate([WKT[:, h * DH:(h + 1) * DH],
                                    WQT[:, h * DH:(h + 1) * DH]], axis=1)
                    for h in range(HEADS)]).astype(np.float32)
    bq, bk, bv = inputs["bq"], inputs["bk"], inputs["bv"]
    BKQ = np.stack([np.concatenate([bk[h * DH:(h + 1) * DH], bq[h * DH:(h + 1) * DH]])
                    for h in range(HEADS)]).astype(np.float32)
    BQ = bq.reshape(HEADS, DH).astype(np.float32)
    BVR = bv.reshape(1, C).astype(np.float32)
    GB1 = np.stack([inputs["g1"], inputs["b1"]]).astype(np.float32)
    GB2 = np.stack([inputs["g2"], inputs["b2"]]).astype(np.float32)
    GB3 = np.stack([inputs["g3"], inputs["b3"]]).astype(np.float32)

    in_maps = []
    for c in range(N_CORES):
        b, s = c // 2, c % 2
        xb = np.roll(x[b], -s * NQ, axis=1).astype(ml_dtypes.bfloat16)
        # tiled layout: contiguous [128, sz] blocks (1 DMA descriptor each)
        xt = np.empty(CIN * N, ml_dtypes.bfloat16)
        for g, (gbase, gw, _subs) in enumerate(XGRP):
            for k in range(8):
                o = XOFF[(g, k)]
                xt[o:o + 128 * gw] = xb[k * 128:(k + 1) * 128,
                                        gbase:gbase + gw].reshape(-1)
        relc = np.ascontiguousarray(rel[:, :, s * NQ:(s + 1) * NQ])
        in_maps.append({
            "X": xt, "W1T": W1T, "WQT": WQT,
            "WVT": WVT, "W3T": W3T, "WKQ": WKQ, "REL": relc, "BKQ": BKQ,
            "BQ": BQ, "BVR": BVR, "GB1": GB1, "GB2": GB2, "GB3": GB3,
        })
    return in_maps


def run(inputs, trace=False, trace_kwargs=None):
    from concourse import bass_utils
    nc = _get_program()
    in_maps = _host_prep(inputs)
    res = bass_utils.run_bass_kernel_spmd(
        nc, in_maps, core_ids=list(range(N_CORES)), trace=trace,
        **(trace_kwargs or {}))
    out = np.empty((B, CIN, N), np.float32)
    for c in range(N_CORES):
        b, s = c // 2, c % 2
        flat = np.asarray(res.results[c]["OUT"]).reshape(-1)
        oc = np.empty((CIN, NQ), np.float32)
        for mt in range(8):
            for cg in range(3):
                off, sz = CGX[cg]
                o = OOFF[(mt, cg)]
                oc[mt * 128:(mt + 1) * 128, off:off + sz] = \
                    flat[o:o + 128 * sz].reshape(128, sz)
        out[b, :, s * NQ:(s + 1) * NQ] = oc
    return out.reshape(B, CIN, 14, 14, 14), res


def kernel(**inputs):
    out, _ = run(inputs, trace=False)
    return out


# revision 33
# speedup vs baseline: 1.0781x; 1.0781x over previous
"""Bottleneck-MHSA fused kernel for 8 Trainium2 NeuronCores (v2).

Sharding: core c = 2*b + s handles batch b; attention queries are split in
half between the two cores of a pair. Each core computes conv1 + BN1 + qkv
for its whole batch (redundantly with its pair partner), then attention for
all 4 heads over its query half, then BN2 + W3 + BN3 + residual for its
query half. Cross-core traffic is only three tiny BN-statistics AllReduces.

v2 changes vs v1 (531us):
- conv1 runs in bf16 (X + W1 host-cast) halving the X DMA that bounded
  phase 1; the residual reuses the resident bf16 X tiles (no re-load).
- BN sum/sumsq stats ride ScalarE activation accum_out (Copy / Square)
  instead of DVE reduce chains; BN applies are single fused
  Relu(scale,bias) activations split across Scalar/DVE/GpSimd.
- Softmax exp is split per query-group: ScalarE does real Exp on ~37%,
  GpSimd+DVE compute Schraudolph exp (one tensor_scalar into an int32
  bitcast) on the rest.  rel-err budget measured on CPU: ~8e-3 << 2e-2.
- Attention post-processing (1/denom broadcast-mult + BN2 stats) moved to
  GpSimd/DVE with double-buffered PSUM accumulators so head/group
  transitions no longer stall the PE (HAM stays warm).
- AllGather+local-reduce -> AllReduce; sqrt via DVE Newton rsqrt so the
  ScalarE activation table never leaves Exp.
- Dummy accumulate-matmul chains keep the PE clock at 2.4GHz through the
  BN1/BN2 collective windows.
"""
import numpy as np
import ml_dtypes

HEADS = 4
DH = 64
C = 256          # PLANES
CIN = 1024       # IN_PLANES
N = 2744         # tokens per batch
NQ = N // 2      # query half per core
B = 4
EPS = 1e-5
N_CORES = 8
CNT12 = 8 * N    # BN1 effective count (pairs double-count; mean/var exact)
CNT2 = 8 * NQ    # BN2/BN3 count (distinct shards)

SCHR_A = 12102203.161561485   # 2^23 / ln 2
SCHR_B = 1064986823.0         # 127*2^23 - 60801*8 (Schraudolph bias)
SCHR_A16 = SCHR_A / 65536.0   # bf16 = top 16 bits of f32
SCHR_B16 = SCHR_B / 65536.0


def _chunks(total, n):
    # even sizes (fp32r matmul requires an even moving free dim)
    assert total % 2 == 0
    half = total // 2
    sizes = [2 * (half // n + (1 if i < half % n else 0)) for i in range(n)]
    out, off = [], 0
    for s in sizes:
        out.append((off, s))
        off += s
    return out


# conv col groups (bf16 X tiles); first 3 cover the residual half 0:1372
CGX = [(0, 512), (512, 512), (1024, 348), (1372, 512), (1884, 512), (2396, 348)]
CH6 = _chunks(N, 6)     # key/token chunks for qkv (456/458 wide)
CH3 = _chunks(NQ, 3)    # query chunks for qkv / W3
MT22 = [(t * 128, min(128, N - t * 128)) for t in range((N + 127) // 128)]
WARM_K = 44             # dummy matmuls per keep-warm chain (~9.5us)

# X DMA groups: (col offset, width, conv sub-splits); first two resident
XGRP = [(0, 1024, [(0, 512), (512, 512)]), (1024, 348, [(0, 348)]),
        (1372, 1024, [(0, 512), (512, 512)]), (2396, 348, [(0, 348)])]
# tiled DRAM offsets: X is staged host-side as contiguous [128, w] blocks
XOFF = {}
_o = 0
for _g, (_off, _w, _) in enumerate(XGRP):
    for _k in range(8):
        XOFF[(_g, _k)] = _o
        _o += 128 * _w
OOFF = {}
_o = 0
for _mt in range(8):
    for _cg in range(3):
        OOFF[(_mt, _cg)] = _o
        _o += 128 * CGX[_cg][1]


def build_program():
    from concourse import bacc, mybir, tile

    F32 = mybir.dt.float32
    F32R = mybir.dt.float32r
    BF16 = mybir.dt.bfloat16
    I32 = mybir.dt.int32

    nc = bacc.Bacc("TRN2", target_bir_lowering=False, debug=False,
                   num_devices=N_CORES)

    # ---- I/O ----
    io = {}
    io["X"] = nc.dram_tensor("X", [CIN * N], BF16, kind="ExternalInput").ap()
    io["W1T"] = nc.dram_tensor("W1T", [CIN, C], BF16, kind="ExternalInput").ap()
    io["WQT"] = nc.dram_tensor("WQT", [C, C], F32R, kind="ExternalInput").ap()
    io["WVT"] = nc.dram_tensor("WVT", [C, C], F32R, kind="ExternalInput").ap()
    io["W3T"] = nc.dram_tensor("W3T", [C, CIN], F32R, kind="ExternalInput").ap()
    io["WKQ"] = nc.dram_tensor("WKQ", [HEADS, C, 128], F32R, kind="ExternalInput").ap()
    io["REL"] = nc.dram_tensor("REL", [HEADS, DH, NQ], F32R, kind="ExternalInput").ap()
    io["BKQ"] = nc.dram_tensor("BKQ", [HEADS, 128], F32, kind="ExternalInput").ap()
    io["BQ"] = nc.dram_tensor("BQ", [HEADS, DH], F32, kind="ExternalInput").ap()
    io["BVR"] = nc.dram_tensor("BVR", [1, C], F32R, kind="ExternalInput").ap()
    io["GB1"] = nc.dram_tensor("GB1", [2, C], F32, kind="ExternalInput").ap()
    io["GB2"] = nc.dram_tensor("GB2", [2, C], F32, kind="ExternalInput").ap()
    io["GB3"] = nc.dram_tensor("GB3", [2, CIN], F32, kind="ExternalInput").ap()
    io["OUT"] = nc.dram_tensor("OUT", [CIN * NQ], F32, kind="ExternalOutput").ap()

    with tile.TileContext(nc) as tc:
        _emit(nc, tc, mybir, F32, F32R, BF16, I32, io)

    nc.compile()
    from concourse.bass_interp import get_hw_module
    nc.m = get_hw_module(nc.m)
    return nc


def _emit(nc, tc, mybir, F32, F32R, BF16, I32, io):
    I16 = mybir.dt.int16
    import contextlib

    AX = mybir.AluOpType
    AF = mybir.ActivationFunctionType
    X_AXIS = mybir.AxisListType.X

    Xd, W1T, WQT, WVT, W3T = io["X"], io["W1T"], io["WQT"], io["WVT"], io["W3T"]
    WKQ, RELd, BKQ, BQd, BVR = io["WKQ"], io["REL"], io["BKQ"], io["BQ"], io["BVR"]
    GB1, GB2, GB3, OUTd = io["GB1"], io["GB2"], io["GB3"], io["OUT"]

    def stats_allreduce(src_sbuf, width, out_sbuf, tag):
        """Sum [P, width] stats over all 8 cores into out_sbuf (AllGather +
        local reduce; measured 2x faster than the AllReduce collective for
        these tiny payloads)."""
        p = src_sbuf.shape[0]
        cin = dpool.tile([p, width], F32, name=f"arin_{tag}")
        cout = dpool.tile([N_CORES, p, width], F32, addr_space="Shared",
                          name=f"arout_{tag}")
        nc.sync.dma_start(cin[:], src_sbuf[:])
        nc.gpsimd.collective_compute(
            "AllGather", AX.bypass,
            replica_groups=[list(range(N_CORES))],
            ins=[cin.opt()], outs=[cout.opt()],
        )
        gath = wpool.tile([p, width, N_CORES], F32, name=f"gath_{tag}")
        nc.sync.dma_start(gath[:], cout[:].rearrange("g p c -> p c g"))
        nc.vector.reduce_sum(out_sbuf[:], gath[:], X_AXIS)

    def rsqrt_newton(y, x, tag):
        """y = 1/sqrt(x) on DVE only (bit-trick seed + 2 Newton steps)."""
        p, w = x.shape[0], x.shape[1]
        xi = x[:].bitcast(I32)
        t1 = wpool.tile([p, w], I32, name=f"rsq_t1_{tag}")
        nc.vector.tensor_scalar(t1[:], xi, 1, None, AX.arith_shift_right)
        yi = y[:].bitcast(I32)
        nc.vector.tensor_scalar(yi, t1[:], -1, 0x5f3759df, AX.mult, AX.add)
        h = wpool.tile([p, w], F32, name=f"rsq_h_{tag}")
        for _ in range(2):
            nc.vector.tensor_tensor(h[:], y[:], y[:], AX.mult)
            nc.vector.tensor_tensor(h[:], x[:], h[:], AX.mult)
            nc.vector.tensor_scalar(h[:], h[:], -0.5, 1.5, AX.mult, AX.add)
            nc.vector.tensor_tensor(y[:], y[:], h[:], AX.mult)

    def bn_coeffs(tot, gt, bt, cnt, w, sc, cc, tag):
        """tot [P, 2w] = [sums | sumsqs] -> scale sc [P, w], bias cc [P, w].
        All DVE (Newton rsqrt) so ScalarE keeps its Exp table loaded."""
        p = tot.shape[0]
        mean = wpool.tile([p, w], F32, name=f"mean_{tag}")
        var = wpool.tile([p, w], F32, name=f"var_{tag}")
        nc.vector.tensor_scalar_mul(mean[:], tot[:, 0:w], 1.0 / cnt)
        nc.vector.tensor_scalar_mul(var[:], tot[:, w:2 * w], 1.0 / cnt)
        m2 = wpool.tile([p, w], F32, name=f"m2_{tag}")
        nc.vector.tensor_tensor(m2[:], mean[:], mean[:], AX.mult)
        nc.vector.tensor_tensor(var[:], var[:], m2[:], AX.subtract)
        nc.vector.tensor_scalar_add(var[:], var[:], EPS)
        rstd = wpool.tile([p, w], F32, name=f"rstd_{tag}")
        rsqrt_newton(rstd, var, tag)
        nc.vector.tensor_tensor(sc[:], gt[:], rstd[:], AX.mult)
        tmp = wpool.tile([p, w], F32, name=f"tmpc_{tag}")
        nc.vector.tensor_tensor(tmp[:], sc[:], mean[:], AX.mult)
        nc.vector.tensor_tensor(cc[:], bt[:], tmp[:], AX.subtract)

    with contextlib.ExitStack() as top:
        wpool = top.enter_context(tc.tile_pool(name="wpool", bufs=1))
        dpool = top.enter_context(tc.tile_pool(name="dpool", bufs=1, space="DRAM"))

        # ---- weights / constants ----
        w1t = []
        for k in range(8):
            t = wpool.tile([128, C], BF16, name=f"w1t{k}")
            nc.scalar.dma_start(t[:], W1T[k * 128:(k + 1) * 128, :])
            w1t.append(t)
        wqt, wvt = [], []
        for srcw, dst, nm in ((WQT, wqt, "wq"), (WVT, wvt, "wv")):
            for k in range(2):
                t = wpool.tile([128, C], F32R, name=f"{nm}{k}")
                nc.scalar.dma_start(t[:], srcw[k * 128:(k + 1) * 128, :])
                dst.append(t)
        wkqt = []
        for h in range(HEADS):
            row = []
            for k in range(2):
                t = wpool.tile([128, 128], F32R, name=f"wkq{h}_{k}")
                nc.scalar.dma_start(t[:], WKQ[h][k * 128:(k + 1) * 128, :])
                row.append(t)
            wkqt.append(row)
        w3t = []
        for k in range(2):
            t = wpool.tile([128, CIN], F32R, name=f"w3t{k}")
            nc.scalar.dma_start(t[:], W3T[k * 128:(k + 1) * 128, :])
            w3t.append(t)
        bvrow = wpool.tile([1, C], F32R, name="bvrow")
        nc.scalar.dma_start(bvrow[:], BVR[:])

        bkqt = wpool.tile([128, HEADS], F32, name="bkqt")
        nc.scalar.dma_start(bkqt[:], BKQ[:].rearrange("h p -> p h"))
        bqt = wpool.tile([DH, HEADS], F32, name="bqt")
        nc.scalar.dma_start(bqt[:], BQd[:].rearrange("h d -> d h"))
        g1t = wpool.tile([128, 2], F32, name="g1t")
        b1t = wpool.tile([128, 2], F32, name="b1t")
        nc.scalar.dma_start(g1t[:], GB1[0].rearrange("(m p) -> p m", p=128))
        nc.scalar.dma_start(b1t[:], GB1[1].rearrange("(m p) -> p m", p=128))
        g2t = wpool.tile([DH, HEADS], F32, name="g2t")
        b2t = wpool.tile([DH, HEADS], F32, name="b2t")
        nc.scalar.dma_start(g2t[:], GB2[0].rearrange("(h d) -> d h", d=DH))
        nc.scalar.dma_start(b2t[:], GB2[1].rearrange("(h d) -> d h", d=DH))
        g3t = wpool.tile([128, 8], F32, name="g3t")
        b3t = wpool.tile([128, 8], F32, name="b3t")
        nc.scalar.dma_start(g3t[:], GB3[0].rearrange("(m p) -> p m", p=128))
        nc.scalar.dma_start(b3t[:], GB3[1].rearrange("(m p) -> p m", p=128))

        # constants: ones pad (bitcast to f32r where needed), warm-chain srcs
        onespad = wpool.tile([128, 128], F32, name="onespad")
        nc.vector.memset(onespad[:], 1.0)
        warmx = wpool.tile([128, 512], F32, name="warmx")
        nc.vector.memset(warmx[:], 0.0)

        def warm_chain(pool, tag, shape, bufs=1, k=WARM_K):
            """Dummy accumulate-matmul chain: keeps the PE clock warm during
            a collective window.  Allocates from the given live PSUM pool."""
            wps = pool.tile(shape, F32, name=f"warm_{tag}", tag=tag, bufs=bufs)
            for i in range(k):
                nc.tensor.matmul(wps[0:128, 0:512], onespad[:].bitcast(F32R),
                                 warmx[:].bitcast(F32R),
                                 start=(i == 0), stop=(i == k - 1))

        # stats accumulators
        S1 = wpool.tile([128, 12], F32, name="S1")   # conv1 sums   (mt*6+cg)
        Q1 = wpool.tile([128, 12], F32, name="Q1")   # conv1 sumsqs
        S2 = wpool.tile([DH, 12], F32, name="S2")    # attn sums    (h*3+gi)
        Q2 = wpool.tile([DH, 12], F32, name="Q2")
        S3 = wpool.tile([128, 24], F32, name="S3")   # W3 sums      (mt*3+ci)
        Q3 = wpool.tile([128, 24], F32, name="Q3")

        # resident bf16 X tiles covering cols 0:1372 (conv input + residual)
        XAB = [wpool.tile([128, 1024], BF16, name=f"xab{k}") for k in range(8)]
        XC = [wpool.tile([128, 348], BF16, name=f"xc{k}") for k in range(8)]
        # residual slices per finale chunk cg in 0..2
        XRES = [[XAB[k][:, 0:512] for k in range(8)],
                [XAB[k][:, 512:1024] for k in range(8)],
                [XC[k][:] for k in range(8)]]
        OUT2 = [wpool.tile([128, NQ], F32R, name=f"out2_{m}") for m in range(2)]

        with contextlib.ExitStack() as ph_a:
            qpool = ph_a.enter_context(tc.tile_pool(name="qpool", bufs=1))
            KHAT = [qpool.tile([128, N], F32R, name=f"khat{h}") for h in range(HEADS)]
            QHAT = [qpool.tile([128, NQ], F32R, name=f"qhat{h}") for h in range(HEADS)]
            # per-head blocks of 22*65 cols: [v^T (64) | ones] per token tile
            VTON = qpool.tile([128, HEADS * 22 * 65], F32R, name="vton")
            # bf16 shadow for the tail-query AV matmuls (Schraudolph path)
            VTONB = qpool.tile([128, HEADS * 22 * 65], BF16, name="vtonb")
            # ones columns (softmax denominator rows), one strided copy per head
            for vt in (VTON, VTONB):
                for h in range(HEADS):
                    dst = vt[:, h * 1430:(h + 1) * 1430].rearrange(
                        "p (t c) -> p t c", t=22)[:, :, DH:DH + 1]
                    src = onespad[:, 0:22].rearrange("p (t c) -> p t c", t=22)
                    nc.vector.tensor_copy(dst, src)
            for h in range(HEADS):
                nc.scalar.dma_start(QHAT[h][DH:128, :], RELd[h])

            with contextlib.ExitStack() as ph1:
                y1pool = ph1.enter_context(tc.tile_pool(name="y1pool", bufs=1))
                Y1 = [y1pool.tile([128, N], F32R, name=f"y1_{m}") for m in range(2)]
                sq1pool = ph1.enter_context(tc.tile_pool(name="sq1pool", bufs=2))

                # ---- phase 1: conv1 (y1 = W1 @ x) bf16, stats on ScalarE ----
                with tc.tile_pool(name="xbpool", bufs=1) as xbpool, \
                     tc.tile_pool(name="psum1", bufs=4, space="PSUM") as psum1:
                    # warm the PE clock while weights/X stream in
                    warm_chain(psum1, "warm0", [128, 512], k=16)
                    dma_engs = (nc.sync, nc.gpsimd)
                    cstat = 0
                    for g, (gbase, gw, subs) in enumerate(XGRP):
                        if g == 0:
                            xts = XAB
                        elif g == 1:
                            xts = XC
                        else:
                            pfx = "xb" if g == 2 else "xd"
                            xts = [xbpool.tile([128, gw], BF16,
                                               name=f"{pfx}{g}_{k}",
                                               tag=f"{pfx}{k}")
                                   for k in range(8)]
                        for k in range(8):
                            xo = XOFF[(g, k)]
                            dma_engs[k % 2].dma_start(
                                xts[k][:], Xd[xo:xo + 128 * gw].rearrange(
                                    "(p f) -> p f", p=128))
                        for so, ssz in subs:
                            off = gbase + so
                            for mt in range(2):
                                ps = psum1.tile([128, ssz], F32, name="pconv",
                                                tag="pconv",
                                                padded_shape=[128, 512])
                                for k in range(8):
                                    nc.tensor.matmul(
                                        ps[:], w1t[k][:, mt * 128:(mt + 1) * 128],
                                        xts[k][:, so:so + ssz],
                                        start=(k == 0), stop=(k == 7))
                                idx = mt * 6 + cstat
                                nc.scalar.activation(Y1[mt][:, off:off + ssz],
                                                     ps[:], AF.Copy,
                                                     accum_out=S1[:, idx:idx + 1])
                                sq = sq1pool.tile([128, ssz], F32, name="sqs",
                                                  tag="sqs",
                                                  padded_shape=[128, 512])
                                nc.scalar.activation(sq[:], ps[:], AF.Square,
                                                     accum_out=Q1[:, idx:idx + 1])
                            cstat += 1

                # ---- phase 1b: BN1 AllReduce + coeffs (DVE); PE keeps warm ----
                s1sum = wpool.tile([128, 2], F32, name="s1sum")
                q1sum = wpool.tile([128, 2], F32, name="q1sum")
                nc.vector.reduce_sum(s1sum[:], S1[:].rearrange("p (m c) -> p m c", c=6), X_AXIS)
                nc.vector.reduce_sum(q1sum[:], Q1[:].rearrange("p (m c) -> p m c", c=6), X_AXIS)
                st1 = wpool.tile([128, 4], F32, name="st1")
                nc.vector.tensor_copy(st1[:, 0:2], s1sum[:])
                nc.vector.tensor_copy(st1[:, 2:4], q1sum[:])
                tot1 = wpool.tile([128, 4], F32, name="tot1")
                stats_allreduce(st1, 4, tot1, "bn1")
                s1c = wpool.tile([128, 2], F32, name="s1c")
                c1c = wpool.tile([128, 2], F32, name="c1c")
                bn_coeffs(tot1, g1t, b1t, CNT12, 2, s1c, c1c, "bn1")

                # ---- phase 2: out1 = relu(s*y1+c) in place, 3-engine split ----
                OUT1 = [Y1[m][:] for m in range(2)]
                for mt in range(2):
                    for cg, (off, sz) in enumerate(CGX):
                        ap = Y1[mt][:, off:off + sz]
                        if cg < 4:
                            nc.scalar.activation(ap, ap, AF.Relu,
                                                 bias=c1c[:, mt:mt + 1],
                                                 scale=s1c[:, mt:mt + 1])
                        else:
                            nc.vector.tensor_scalar(ap, ap,
                                                    s1c[:, mt:mt + 1],
                                                    c1c[:, mt:mt + 1],
                                                    AX.mult, AX.add)
                            nc.vector.tensor_scalar(ap, ap, 0.0, None, AX.max)

                # ---- phase 3a: vT = out1^T @ WvT (+bv), into VTON (ScalarE) ----
                with tc.tile_pool(name="psum3", bufs=1, space="PSUM") as psum3:
                    # keep the PE clock warm through the BN1 collective window
                    warm_chain(psum3, "warm1", [128, 512])
                    for t, (mo, msz) in enumerate(MT22):
                        ps = psum3.tile([128, C], F32, name="pvt", tag="pvt", bufs=2)
                        nc.tensor.matmul(ps[0:msz, :],
                                         onespad[0:1, 0:msz].bitcast(F32R),
                                         bvrow[:], start=True, stop=False)
                        for k in range(2):
                            nc.tensor.matmul(ps[0:msz, :], OUT1[k][:, mo:mo + msz],
                                             wvt[k][:], start=False, stop=(k == 1))
                        # one strided copy for all 4 heads: [msz, 4, 64]
                        dst = VTON[0:msz, :].rearrange(
                            "p (h c) -> p h c", h=HEADS)[:, :, t * 65:t * 65 + DH]
                        src = ps[0:msz, :].rearrange("p (h c) -> p h c", h=HEADS)
                        nc.scalar.activation(dst, src, AF.Copy)
                        dstb = VTONB[0:msz, :].rearrange(
                            "p (h c) -> p h c", h=HEADS)[:, :, t * 65:t * 65 + DH]
                        nc.vector.tensor_copy(dstb, src)

                    # ---- phase 3b: KHAT = [k;q] packed, QHAT q-half ----
                    # bias adds split Scalar (Identity+bias) / DVE; GpSimd
                    # cannot read PSUM.
                    for h in range(HEADS):
                        hs = h * DH
                        for ci, (off, sz) in enumerate(CH6):
                            ps = psum3.tile([128, sz], F32, name="pkh", tag="pkh",
                                            bufs=3)
                            for k in range(2):
                                nc.tensor.matmul(ps[:], wkqt[h][k][:],
                                                 OUT1[k][:, off:off + sz],
                                                 start=(k == 0), stop=(k == 1))
                            if ci % 2 == 0:
                                nc.scalar.activation(KHAT[h][:, off:off + sz],
                                                     ps[:], AF.Identity,
                                                     bias=bkqt[:, h:h + 1])
                            else:
                                nc.vector.tensor_scalar(KHAT[h][:, off:off + sz],
                                                        ps[:], bkqt[:, h:h + 1],
                                                        None, AX.add)
                        for (off, sz) in CH3:
                            pq = psum3.tile([DH, sz], F32, name="pqh", tag="pqh",
                                            bufs=2)
                            for k in range(2):
                                nc.tensor.matmul(pq[:], wqt[k][:, hs:hs + DH],
                                                 OUT1[k][:, off:off + sz],
                                                 start=(k == 0), stop=(k == 1))
                            nc.vector.tensor_scalar(QHAT[h][0:DH, off:off + sz], pq[:],
                                                    bqt[:, h:h + 1], None, AX.add)

            # ---- phase 4: attention ----
            # queries 0:1024 per head run as a Scalar-dense 1024-wide loop;
            # the 348-query tails of two heads are interleaved afterwards so
            # their serial logits->exp->AV chains overlap.  The softmax
            # denominator reciprocal is exp(-ln(d)) on ScalarE (a [1,512]
            # DVE reciprocal costs 3us; ScalarE does the pair in 1.1us).
            with tc.tile_pool(name="oattp", bufs=1) as oattp, \
                 tc.tile_pool(name="epool", bufs=1) as epool:
                OATT = [oattp.tile([DH, NQ], F32R, name=f"oatt{h}")
                        for h in range(HEADS)]

                def post_group(pool, pav, h, off, sz, idx, pbtag, pbbufs):
                    rcr = epool.tile([1, sz], F32, name="rcr", tag="rcr",
                                     bufs=2, padded_shape=[1, 512])
                    nc.vector.reciprocal(rcr[:], pav[DH:65, :])
                    pb = pool.tile([DH, sz], F32, name="pb", tag=pbtag,
                                   bufs=pbbufs, padded_shape=[DH, 512])
                    nc.tensor.matmul(pb[:], onespad[0:1, 0:DH], rcr[:],
                                     start=True, stop=True)
                    pbs = epool.tile([DH, sz], F32, name="pbs", tag="pbs",
                                     bufs=2, padded_shape=[DH, 512])
                    nc.vector.tensor_copy(pbs[:], pb[:])
                    nc.vector.scalar_tensor_tensor(
                        OATT[h][:, off:off + sz], pav[0:DH, :], 1.0, pbs[:],
                        AX.mult, AX.mult, accum_out=S2[:, idx:idx + 1])
                    sq2 = epool.tile([DH, sz], F32, name="sq2", tag="sq2",
                                     bufs=2, padded_shape=[DH, 512])
                    nc.vector.scalar_tensor_tensor(
                        sq2[:], OATT[h][:, off:off + sz], 1.0,
                        OATT[h][:, off:off + sz],
                        AX.mult, AX.mult, accum_out=Q2[:, idx:idx + 1])

                with tc.tile_pool(name="psum4a", bufs=1, space="PSUM") as ps4a:
                    for h in range(HEADS):
                        pavs = [ps4a.tile([65, 512], F32, name=f"pav{si}",
                                          tag=f"pav{si}", bufs=1)
                                for si in range(2)]
                        for t, (mo, msz) in enumerate(MT22):
                            ps = ps4a.tile([128, 1024], F32, name="ps", tag="ps",
                                           bufs=2)
                            for so in (0, 512):
                                nc.tensor.matmul(ps[0:msz, so:so + 512],
                                                 KHAT[h][:, mo:mo + msz],
                                                 QHAT[h][:, so:so + 512],
                                                 start=True, stop=True)
                            e = epool.tile([128, 1024], F32R, name="e", tag="e",
                                           bufs=3)
                            nc.scalar.activation(e[0:msz, :], ps[0:msz, :],
                                                 AF.Exp)
                            base = (h * 22 + t) * 65
                            for si, so in enumerate((0, 512)):
                                nc.tensor.matmul(pavs[si][:],
                                                 VTON[0:msz, base:base + 65],
                                                 e[0:msz, so:so + 512],
                                                 start=(t == 0), stop=(t == 21))
                        for si in range(2):
                            post_group(ps4a, pavs[si], h, si * 512, 512,
                                       h * 3 + si, "pb", 1)

                with tc.tile_pool(name="psum4b", bufs=1, space="PSUM") as ps4b:
                    for pair in ((0, 1), (2, 3)):
                        pav2 = {h: ps4b.tile([65, 348], F32, name=f"pav2_{h}",
                                             tag=f"pav2_{h % 2}", bufs=1,
                                             padded_shape=[65, 512])
                                for h in pair}
                        for t, (mo, msz) in enumerate(MT22):
                            for h in pair:
                                ps = ps4b.tile([128, 348], F32, name="ps2",
                                               tag="ps2", bufs=4,
                                               padded_shape=[128, 512])
                                nc.tensor.matmul(ps[0:msz, :],
                                                 KHAT[h][:, mo:mo + msz],
                                                 QHAT[h][:, 1024:1372],
                                                 start=True, stop=True)
                                e2 = epool.tile([128, 348], BF16, name="e2",
                                                tag="e2", bufs=4,
                                                padded_shape=[128, 512])
                                nc.vector.tensor_scalar(
                                    e2[0:msz, :].bitcast(I16), ps[0:msz, :],
                                    SCHR_A16, SCHR_B16, AX.mult, AX.add)
                                base = (h * 22 + t) * 65
                                nc.tensor.matmul(pav2[h][:],
                                                 VTONB[0:msz, base:base + 65],
                                                 e2[0:msz, :],
                                                 start=(t == 0), stop=(t == 21))
                        for h in pair:
                            post_group(ps4b, pav2[h], h, 1024, 348,
                                       h * 3 + 2, "pb2", 2)
                            # move raw head output into OUT2 layout
                            nc.sync.dma_start(
                                OUT2[h // 2][(h % 2) * DH:(h % 2) * DH + DH, :],
                                OATT[h][:])

                # ---- phase 5: BN2 AllReduce + fused Relu apply ----
                s2sum = wpool.tile([DH, 4], F32, name="s2sum")
                q2sum = wpool.tile([DH, 4], F32, name="q2sum")
                nc.vector.reduce_sum(s2sum[:], S2[:].rearrange("p (h c) -> p h c", c=3), X_AXIS)
                nc.vector.reduce_sum(q2sum[:], Q2[:].rearrange("p (h c) -> p h c", c=3), X_AXIS)
                st2 = wpool.tile([DH, 8], F32, name="st2")
                nc.vector.tensor_copy(st2[:, 0:4], s2sum[:])
                nc.vector.tensor_copy(st2[:, 4:8], q2sum[:])
                tot2 = wpool.tile([DH, 8], F32, name="tot2")
                stats_allreduce(st2, 8, tot2, "bn2")
                s2c = wpool.tile([DH, 4], F32, name="s2c")
                c2c = wpool.tile([DH, 4], F32, name="c2c")
                bn_coeffs(tot2, g2t, b2t, CNT2, 4, s2c, c2c, "bn2")
                # rearrange [64,4] head coeffs -> [128,2] OUT2 channel layout
                s2c128 = wpool.tile([128, 2], F32, name="s2c128")
                c2c128 = wpool.tile([128, 2], F32, name="c2c128")
                for s in range(2):
                    nc.sync.dma_start(
                        s2c128[s * DH:(s + 1) * DH, :],
                        s2c[:].rearrange("d (m s) -> d m s", s=2)[:, :, s])
                    nc.sync.dma_start(
                        c2c128[s * DH:(s + 1) * DH, :],
                        c2c[:].rearrange("d (m s) -> d m s", s=2)[:, :, s])
                for m in range(2):
                    nc.scalar.activation(OUT2[m][:], OUT2[m][:], AF.Relu,
                                         bias=c2c128[:, m:m + 1],
                                         scale=s2c128[:, m:m + 1])

        # ---- phase 6: y3 = W3 @ out2; stats Scalar+DVE; finale 3-engine ----
        with tc.tile_pool(name="y3pool", bufs=1) as y3pool, \
             tc.tile_pool(name="fpool", bufs=2) as fpool, \
             tc.tile_pool(name="psum6", bufs=3, space="PSUM") as psum6:
            # keep the PE clock warm through the BN2 collective window
            warm_chain(psum6, "warm2", [128, 512], bufs=1)
            Y3 = [y3pool.tile([128, NQ], BF16, name=f"y3_{m}") for m in range(8)]
            for mt in range(8):
                for ci, (off, sz) in enumerate(CH3):
                    ps = psum6.tile([128, sz], F32, name="pw3", tag="pw3")
                    for k in range(2):
                        nc.tensor.matmul(ps[:], w3t[k][:, mt * 128:(mt + 1) * 128],
                                         OUT2[k][:, off:off + sz],
                                         start=(k == 0), stop=(k == 1))
                    idx = mt * 3 + ci
                    nc.scalar.activation(Y3[mt][:, off:off + sz], ps[:], AF.Copy,
                                         accum_out=S3[:, idx:idx + 1])
                    sq = fpool.tile([128, sz], F32, name="sq3", tag="sq3")
                    nc.vector.scalar_tensor_tensor(
                        sq[:], Y3[mt][:, off:off + sz], 1.0,
                        Y3[mt][:, off:off + sz], AX.mult, AX.mult,
                        accum_out=Q3[:, idx:idx + 1])

            s3sum = wpool.tile([128, 8], F32, name="s3sum")
            q3sum = wpool.tile([128, 8], F32, name="q3sum")
            nc.vector.reduce_sum(s3sum[:], S3[:].rearrange("p (m c) -> p m c", c=3), X_AXIS)
            nc.vector.reduce_sum(q3sum[:], Q3[:].rearrange("p (m c) -> p m c", c=3), X_AXIS)
            st3 = wpool.tile([128, 16], F32, name="st3")
            nc.vector.tensor_copy(st3[:, 0:8], s3sum[:])
            nc.vector.tensor_copy(st3[:, 8:16], q3sum[:])
            tot3 = wpool.tile([128, 16], F32, name="tot3")
            stats_allreduce(st3, 16, tot3, "bn3")
            s3c = wpool.tile([128, 8], F32, name="s3c")
            c3c = wpool.tile([128, 8], F32, name="c3c")
            bn_coeffs(tot3, g3t, b3t, CNT2, 8, s3c, c3c, "bn3")

            # finale: out = relu(s3*y3 + c3 + x); op1 on DVE (stt with the
            # resident bf16 X tiles), op2 split Scalar (cg 0,2) / DVE (cg 1)
            for mt in range(8):
                for cg in range(3):
                    off, sz = CGX[cg]
                    tf = fpool.tile([128, sz], F32, name="tf", tag="tf", bufs=3,
                                    padded_shape=[128, 512])
                    nc.vector.scalar_tensor_tensor(
                        tf[:], Y3[mt][:, off:off + sz], s3c[:, mt:mt + 1],
                        XRES[cg][mt], AX.mult, AX.add)
                    to = fpool.tile([128, sz], F32, name="to", tag="to", bufs=3,
                                    padded_shape=[128, 512])
                    if cg != 1:
                        nc.scalar.activation(to[:], tf[:], AF.Relu,
                                             bias=c3c[:, mt:mt + 1])
                    else:
                        nc.vector.tensor_scalar(to[:], tf[:], c3c[:, mt:mt + 1],
                                                0.0, AX.add, AX.max)
                    oo = OOFF[(mt, cg)]
                    nc.sync.dma_start(OUTd[oo:oo + 128 * sz].rearrange(
                        "(p f) -> p f", p=128), to[:])


_NC_CACHE = {}


def _get_program():
    if "nc" not in _NC_CACHE:
        _NC_CACHE["nc"] = build_program()
    return _NC_CACHE["nc"]


def _host_prep(inputs):
    x = np.ascontiguousarray(inputs["x"].reshape(B, CIN, N))
    rel = (inputs["rel_h"] + inputs["rel_w"] + inputs["rel_d"]).reshape(HEADS, DH, N)
    rel = np.ascontiguousarray(rel.astype(np.float32))
    W1T = np.ascontiguousarray(inputs["W1"].T.astype(ml_dtypes.bfloat16))
    WQT = np.ascontiguousarray(inputs["Wq"].T.astype(np.float32))
    WKT = np.ascontiguousarray(inputs["Wk"].T.astype(np.float32))
    WVT = np.ascontiguousarray(inputs["Wv"].T.astype(np.float32))
    W3T = np.ascontiguousarray(inputs["W3"].T.astype(np.float32))
    WKQ = np.stack([np.concatenate([WKT[:, h * DH:(h + 1) * DH],
                                    WQT[:, h * DH:(h + 1) * DH]], axis=1)
                    for h in range(HEADS)]).astype(np.float32)
    bq, bk, bv = inputs["bq"], inputs["bk"], inputs["bv"]
    BKQ = np.stack([np.concatenate([bk[h * DH:(h + 1) * DH], bq[h * DH:(h + 1) * DH]])
                    for h in range(HEADS)]).astype(np.float32)
    BQ = bq.reshape(HEADS, DH).astype(np.float32)
    BVR = bv.reshape(1, C).astype(np.float32)
    GB1 = np.stack([inputs["g1"], inputs["b1"]]).astype(np.float32)
    GB2 = np.stack([inputs["g2"], inputs["b2"]]).astype(np.float32)
    GB3 = np.stack([inputs["g3"], inputs["b3"]]).astype(np.float32)

    in_maps = []
    for c in range(N_CORES):
        b, s = c // 2, c % 2
        xb = np.roll(x[b], -s * NQ, axis=1).astype(ml_dtypes.bfloat16)
        # tiled layout: contiguous [128, sz] blocks (1 DMA descriptor each)
        xt = np.empty(CIN * N, ml_dtypes.bfloat16)
        for g, (gbase, gw, _subs) in enumerate(XGRP):
            for k in range(8):
                o = XOFF[(g, k)]
                xt[o:o + 128 * gw] = xb[k * 128:(k + 1) * 128,
                                        gbase:gbase + gw].reshape(-1)
        relc = np.ascontiguousarray(rel[:, :, s * NQ:(s + 1) * NQ])
        in_maps.append({
            "X": xt, "W1T": W1T, "WQT": WQT,
            "WVT": WVT, "W3T": W3T, "WKQ": WKQ, "REL": relc, "BKQ": BKQ,
            "BQ": BQ, "BVR": BVR, "GB1": GB1, "GB2": GB2, "GB3": GB3,
        })
    return in_maps


def run(inputs, trace=False, trace_kwargs=None):
    from concourse import bass_utils
    nc = _get_program()
    in_maps = _host_prep(inputs)
    res = bass_utils.run_bass_kernel_spmd(
        nc, in_maps, core_ids=list(range(N_CORES)), trace=trace,
        **(trace_kwargs or {}))
    out = np.empty((B, CIN, N), np.float32)
    for c in range(N_CORES):
        b, s = c // 2, c % 2
        flat = np.asarray(res.results[c]["OUT"]).reshape(-1)
        oc = np.empty((CIN, NQ), np.float32)
        for mt in range(8):
            for cg in range(3):
                off, sz = CGX[cg]
                o = OOFF[(mt, cg)]
                oc[mt * 128:(mt + 1) * 128, off:off + sz] = \
                    flat[o:o + 128 * sz].reshape(128, sz)
        out[b, :, s * NQ:(s + 1) * NQ] = oc
    return out.reshape(B, CIN, 14, 14, 14), res


def kernel(**inputs):
    out, _ = run(inputs, trace=False)
    return out


# revision 35
# speedup vs baseline: 1.1356x; 1.0533x over previous
"""Bottleneck-MHSA fused kernel for 8 Trainium2 NeuronCores (v2).

Sharding: core c = 2*b + s handles batch b; attention queries are split in
half between the two cores of a pair. Each core computes conv1 + BN1 + qkv
for its whole batch (redundantly with its pair partner), then attention for
all 4 heads over its query half, then BN2 + W3 + BN3 + residual for its
query half. Cross-core traffic is only three tiny BN-statistics AllReduces.

v2 changes vs v1 (531us):
- conv1 runs in bf16 (X + W1 host-cast) halving the X DMA that bounded
  phase 1; the residual reuses the resident bf16 X tiles (no re-load).
- BN sum/sumsq stats ride ScalarE activation accum_out (Copy / Square)
  instead of DVE reduce chains; BN applies are single fused
  Relu(scale,bias) activations split across Scalar/DVE/GpSimd.
- Softmax exp is split per query-group: ScalarE does real Exp on ~37%,
  GpSimd+DVE compute Schraudolph exp (one tensor_scalar into an int32
  bitcast) on the rest.  rel-err budget measured on CPU: ~8e-3 << 2e-2.
- Attention post-processing (1/denom broadcast-mult + BN2 stats) moved to
  GpSimd/DVE with double-buffered PSUM accumulators so head/group
  transitions no longer stall the PE (HAM stays warm).
- AllGather+local-reduce -> AllReduce; sqrt via DVE Newton rsqrt so the
  ScalarE activation table never leaves Exp.
- Dummy accumulate-matmul chains keep the PE clock at 2.4GHz through the
  BN1/BN2 collective windows.
"""
import numpy as np
import ml_dtypes

HEADS = 4
DH = 64
C = 256          # PLANES
CIN = 1024       # IN_PLANES
N = 2744         # tokens per batch
NQ = N // 2      # query half per core
B = 4
EPS = 1e-5
N_CORES = 8
CNT12 = 8 * N    # BN1 effective count (pairs double-count; mean/var exact)
CNT2 = 8 * NQ    # BN2/BN3 count (distinct shards)

SCHR_A = 12102203.161561485   # 2^23 / ln 2
SCHR_B = 1064986823.0         # 127*2^23 - 60801*8 (Schraudolph bias)
SCHR_A16 = SCHR_A / 65536.0   # bf16 = top 16 bits of f32
SCHR_B16 = SCHR_B / 65536.0


def _chunks(total, n):
    # even sizes (fp32r matmul requires an even moving free dim)
    assert total % 2 == 0
    half = total // 2
    sizes = [2 * (half // n + (1 if i < half % n else 0)) for i in range(n)]
    out, off = [], 0
    for s in sizes:
        out.append((off, s))
        off += s
    return out


# conv col groups (bf16 X tiles); first 3 cover the residual half 0:1372
CGX = [(0, 512), (512, 512), (1024, 348), (1372, 512), (1884, 512), (2396, 348)]
CH6 = _chunks(N, 6)     # key/token chunks for qkv (456/458 wide)
CH3 = _chunks(NQ, 3)    # query chunks for qkv / W3
MT22 = [(t * 128, min(128, N - t * 128)) for t in range((N + 127) // 128)]
WARM_K = 44             # dummy matmuls per keep-warm chain (~9.5us)

# X DMA groups: (col offset, width, conv sub-splits); first two resident
XGRP = [(0, 1024, [(0, 512), (512, 512)]), (1024, 348, [(0, 348)]),
        (1372, 1024, [(0, 512), (512, 512)]), (2396, 348, [(0, 348)])]
# tiled DRAM offsets: X is staged host-side as contiguous [128, w] blocks
XOFF = {}
_o = 0
for _g, (_off, _w, _) in enumerate(XGRP):
    for _k in range(8):
        XOFF[(_g, _k)] = _o
        _o += 128 * _w
OOFF = {}
_o = 0
for _mt in range(8):
    for _cg in range(3):
        OOFF[(_mt, _cg)] = _o
        _o += 128 * CGX[_cg][1]


def build_program():
    from concourse import bacc, mybir, tile

    F32 = mybir.dt.float32
    F32R = mybir.dt.float32r
    BF16 = mybir.dt.bfloat16
    I32 = mybir.dt.int32

    nc = bacc.Bacc("TRN2", target_bir_lowering=False, debug=False,
                   num_devices=N_CORES)

    # ---- I/O ----
    io = {}
    io["X"] = nc.dram_tensor("X", [CIN * N], BF16, kind="ExternalInput").ap()
    io["W1T"] = nc.dram_tensor("W1T", [CIN, C], BF16, kind="ExternalInput").ap()
    io["WQT"] = nc.dram_tensor("WQT", [C, C], F32R, kind="ExternalInput").ap()
    io["WVT"] = nc.dram_tensor("WVT", [C, C], F32R, kind="ExternalInput").ap()
    io["W3T"] = nc.dram_tensor("W3T", [C, CIN], F32R, kind="ExternalInput").ap()
    io["WKQ"] = nc.dram_tensor("WKQ", [HEADS, C, 128], F32R, kind="ExternalInput").ap()
    io["REL"] = nc.dram_tensor("REL", [HEADS, DH, NQ], F32R, kind="ExternalInput").ap()
    io["BKQ"] = nc.dram_tensor("BKQ", [HEADS, 128], F32, kind="ExternalInput").ap()
    io["BQ"] = nc.dram_tensor("BQ", [HEADS, DH], F32, kind="ExternalInput").ap()
    io["BVR"] = nc.dram_tensor("BVR", [1, C], F32R, kind="ExternalInput").ap()
    io["GB1"] = nc.dram_tensor("GB1", [2, C], F32, kind="ExternalInput").ap()
    io["GB2"] = nc.dram_tensor("GB2", [2, C], F32, kind="ExternalInput").ap()
    io["GB3"] = nc.dram_tensor("GB3", [2, CIN], F32, kind="ExternalInput").ap()
    io["OUT"] = nc.dram_tensor("OUT", [CIN * NQ], F32, kind="ExternalOutput").ap()

    with tile.TileContext(nc) as tc:
        _emit(nc, tc, mybir, F32, F32R, BF16, I32, io)

    nc.compile()
    from concourse.bass_interp import get_hw_module
    nc.m = get_hw_module(nc.m)
    return nc


def _emit(nc, tc, mybir, F32, F32R, BF16, I32, io):
    I16 = mybir.dt.int16
    import contextlib

    AX = mybir.AluOpType
    AF = mybir.ActivationFunctionType
    X_AXIS = mybir.AxisListType.X

    Xd, W1T, WQT, WVT, W3T = io["X"], io["W1T"], io["WQT"], io["WVT"], io["W3T"]
    WKQ, RELd, BKQ, BQd, BVR = io["WKQ"], io["REL"], io["BKQ"], io["BQ"], io["BVR"]
    GB1, GB2, GB3, OUTd = io["GB1"], io["GB2"], io["GB3"], io["OUT"]

    def stats_allreduce(src_sbuf, width, out_sbuf, tag):
        """Sum [P, width] stats over all 8 cores into out_sbuf (AllGather +
        local reduce; measured 2x faster than the AllReduce collective for
        these tiny payloads)."""
        p = src_sbuf.shape[0]
        cin = dpool.tile([p, width], F32, name=f"arin_{tag}")
        cout = dpool.tile([N_CORES, p, width], F32, addr_space="Shared",
                          name=f"arout_{tag}")
        nc.sync.dma_start(cin[:], src_sbuf[:])
        nc.gpsimd.collective_compute(
            "AllGather", AX.bypass,
            replica_groups=[list(range(N_CORES))],
            ins=[cin.opt()], outs=[cout.opt()],
        )
        gath = wpool.tile([p, N_CORES, width], F32, name=f"gath_{tag}")
        nc.sync.dma_start(gath[:], cout[:].rearrange("g p c -> p g c"))
        nc.vector.reduce_sum(out_sbuf[:],
                             gath[:].rearrange("p g c -> p c g"), X_AXIS)

    def rsqrt_newton(y, x, tag):
        """y = 1/sqrt(x) on DVE only (bit-trick seed + 2 Newton steps)."""
        p, w = x.shape[0], x.shape[1]
        xi = x[:].bitcast(I32)
        t1 = wpool.tile([p, w], I32, name=f"rsq_t1_{tag}")
        nc.vector.tensor_scalar(t1[:], xi, 1, None, AX.arith_shift_right)
        yi = y[:].bitcast(I32)
        nc.vector.tensor_scalar(yi, t1[:], -1, 0x5f3759df, AX.mult, AX.add)
        h = wpool.tile([p, w], F32, name=f"rsq_h_{tag}")
        for _ in range(2):
            nc.vector.tensor_tensor(h[:], y[:], y[:], AX.mult)
            nc.vector.tensor_tensor(h[:], x[:], h[:], AX.mult)
            nc.vector.tensor_scalar(h[:], h[:], -0.5, 1.5, AX.mult, AX.add)
            nc.vector.tensor_tensor(y[:], y[:], h[:], AX.mult)

    def bn_coeffs(tot, gt, bt, cnt, w, sc, cc, tag):
        """tot [P, 2w] = [sums | sumsqs] -> scale sc [P, w], bias cc [P, w].
        All DVE (Newton rsqrt) so ScalarE keeps its Exp table loaded."""
        p = tot.shape[0]
        mean = wpool.tile([p, w], F32, name=f"mean_{tag}")
        var = wpool.tile([p, w], F32, name=f"var_{tag}")
        nc.vector.tensor_scalar_mul(mean[:], tot[:, 0:w], 1.0 / cnt)
        nc.vector.tensor_scalar_mul(var[:], tot[:, w:2 * w], 1.0 / cnt)
        m2 = wpool.tile([p, w], F32, name=f"m2_{tag}")
        nc.vector.tensor_tensor(m2[:], mean[:], mean[:], AX.mult)
        nc.vector.tensor_tensor(var[:], var[:], m2[:], AX.subtract)
        nc.vector.tensor_scalar_add(var[:], var[:], EPS)
        rstd = wpool.tile([p, w], F32, name=f"rstd_{tag}")
        rsqrt_newton(rstd, var, tag)
        nc.vector.tensor_tensor(sc[:], gt[:], rstd[:], AX.mult)
        tmp = wpool.tile([p, w], F32, name=f"tmpc_{tag}")
        nc.vector.tensor_tensor(tmp[:], sc[:], mean[:], AX.mult)
        nc.vector.tensor_tensor(cc[:], bt[:], tmp[:], AX.subtract)

    with contextlib.ExitStack() as top:
        wpool = top.enter_context(tc.tile_pool(name="wpool", bufs=1))
        dpool = top.enter_context(tc.tile_pool(name="dpool", bufs=1, space="DRAM"))

        # ---- weights / constants ----
        w1t = []
        for k in range(8):
            t = wpool.tile([128, C], BF16, name=f"w1t{k}")
            nc.scalar.dma_start(t[:], W1T[k * 128:(k + 1) * 128, :])
            w1t.append(t)
        wqt, wvt = [], []
        for srcw, dst, nm in ((WQT, wqt, "wq"), (WVT, wvt, "wv")):
            for k in range(2):
                t = wpool.tile([128, C], F32R, name=f"{nm}{k}")
                nc.scalar.dma_start(t[:], srcw[k * 128:(k + 1) * 128, :])
                dst.append(t)
        wkqt = []
        for h in range(HEADS):
            row = []
            for k in range(2):
                t = wpool.tile([128, 128], F32R, name=f"wkq{h}_{k}")
                nc.scalar.dma_start(t[:], WKQ[h][k * 128:(k + 1) * 128, :])
                row.append(t)
            wkqt.append(row)
        w3t = []
        for k in range(2):
            t = wpool.tile([128, CIN], F32R, name=f"w3t{k}")
            nc.scalar.dma_start(t[:], W3T[k * 128:(k + 1) * 128, :])
            w3t.append(t)
        bvrow = wpool.tile([1, C], F32R, name="bvrow")
        nc.scalar.dma_start(bvrow[:], BVR[:])

        bkqt = wpool.tile([128, HEADS], F32, name="bkqt")
        nc.scalar.dma_start(bkqt[:], BKQ[:].rearrange("h p -> p h"))
        bqt = wpool.tile([DH, HEADS], F32, name="bqt")
        nc.scalar.dma_start(bqt[:], BQd[:].rearrange("h d -> d h"))
        g1t = wpool.tile([128, 2], F32, name="g1t")
        b1t = wpool.tile([128, 2], F32, name="b1t")
        nc.scalar.dma_start(g1t[:], GB1[0].rearrange("(m p) -> p m", p=128))
        nc.scalar.dma_start(b1t[:], GB1[1].rearrange("(m p) -> p m", p=128))
        g2t = wpool.tile([DH, HEADS], F32, name="g2t")
        b2t = wpool.tile([DH, HEADS], F32, name="b2t")
        nc.scalar.dma_start(g2t[:], GB2[0].rearrange("(h d) -> d h", d=DH))
        nc.scalar.dma_start(b2t[:], GB2[1].rearrange("(h d) -> d h", d=DH))
        g3t = wpool.tile([128, 8], F32, name="g3t")
        b3t = wpool.tile([128, 8], F32, name="b3t")
        nc.scalar.dma_start(g3t[:], GB3[0].rearrange("(m p) -> p m", p=128))
        nc.scalar.dma_start(b3t[:], GB3[1].rearrange("(m p) -> p m", p=128))

        # constants: ones pad (bitcast to f32r where needed), warm-chain srcs
        onespad = wpool.tile([128, 128], F32, name="onespad")
        nc.vector.memset(onespad[:], 1.0)
        warmx = wpool.tile([128, 512], F32, name="warmx")
        nc.vector.memset(warmx[:], 0.0)

        def warm_chain(pool, tag, shape, bufs=1, k=WARM_K):
            """Dummy accumulate-matmul chain: keeps the PE clock warm during
            a collective window.  Allocates from the given live PSUM pool."""
            wps = pool.tile(shape, F32, name=f"warm_{tag}", tag=tag, bufs=bufs)
            for i in range(k):
                nc.tensor.matmul(wps[0:128, 0:512], onespad[:].bitcast(F32R),
                                 warmx[:].bitcast(F32R),
                                 start=(i == 0), stop=(i == k - 1))

        # stats accumulators
        S1 = wpool.tile([128, 12], F32, name="S1")   # conv1 sums   (mt*6+cg)
        Q1 = wpool.tile([128, 12], F32, name="Q1")   # conv1 sumsqs
        S2 = wpool.tile([DH, 12], F32, name="S2")    # attn sums    (h*3+gi)
        Q2 = wpool.tile([DH, 12], F32, name="Q2")
        S3 = wpool.tile([128, 24], F32, name="S3")   # W3 sums      (mt*3+ci)
        Q3 = wpool.tile([128, 24], F32, name="Q3")

        # resident bf16 X tiles covering cols 0:1372 (conv input + residual)
        XAB = [wpool.tile([128, 1024], BF16, name=f"xab{k}") for k in range(8)]
        XC = [wpool.tile([128, 348], BF16, name=f"xc{k}") for k in range(8)]
        # residual slices per finale chunk cg in 0..2
        XRES = [[XAB[k][:, 0:512] for k in range(8)],
                [XAB[k][:, 512:1024] for k in range(8)],
                [XC[k][:] for k in range(8)]]
        OUT2 = [wpool.tile([128, NQ], F32R, name=f"out2_{m}") for m in range(2)]

        with contextlib.ExitStack() as ph_a:
            qpool = ph_a.enter_context(tc.tile_pool(name="qpool", bufs=1))
            KHAT = [qpool.tile([128, N], F32R, name=f"khat{h}") for h in range(HEADS)]
            QHAT = [qpool.tile([128, NQ], F32R, name=f"qhat{h}") for h in range(HEADS)]
            # per-head blocks of 22*65 cols: [v^T (64) | ones] per token tile
            VTON = qpool.tile([128, HEADS * 22 * 65], F32R, name="vton")
            # bf16 shadow for the tail-query AV matmuls (Schraudolph path)
            VTONB = qpool.tile([128, HEADS * 22 * 65], BF16, name="vtonb")
            # ones columns (softmax denominator rows), one strided copy per head
            for vt in (VTON, VTONB):
                for h in range(HEADS):
                    dst = vt[:, h * 1430:(h + 1) * 1430].rearrange(
                        "p (t c) -> p t c", t=22)[:, :, DH:DH + 1]
                    src = onespad[:, 0:22].rearrange("p (t c) -> p t c", t=22)
                    nc.vector.tensor_copy(dst, src)
            for h in range(HEADS):
                nc.scalar.dma_start(QHAT[h][DH:128, :], RELd[h])

            with contextlib.ExitStack() as ph1:
                y1pool = ph1.enter_context(tc.tile_pool(name="y1pool", bufs=1))
                Y1 = [y1pool.tile([128, N], F32R, name=f"y1_{m}") for m in range(2)]
                sq1pool = ph1.enter_context(tc.tile_pool(name="sq1pool", bufs=2))

                # ---- phase 1: conv1 (y1 = W1 @ x) bf16, stats on ScalarE ----
                with tc.tile_pool(name="xbpool", bufs=1) as xbpool, \
                     tc.tile_pool(name="psum1", bufs=4, space="PSUM") as psum1:
                    # warm the PE clock while weights/X stream in
                    warm_chain(psum1, "warm0", [128, 512], k=16)
                    dma_engs = (nc.sync, nc.gpsimd)
                    cstat = 0
                    for g, (gbase, gw, subs) in enumerate(XGRP):
                        if g == 0:
                            xts = XAB
                        elif g == 1:
                            xts = XC
                        else:
                            pfx = "xb" if g == 2 else "xd"
                            xts = [xbpool.tile([128, gw], BF16,
                                               name=f"{pfx}{g}_{k}",
                                               tag=f"{pfx}{k}")
                                   for k in range(8)]
                        for k in range(8):
                            xo = XOFF[(g, k)]
                            dma_engs[k % 2].dma_start(
                                xts[k][:], Xd[xo:xo + 128 * gw].rearrange(
                                    "(p f) -> p f", p=128))
                        for so, ssz in subs:
                            off = gbase + so
                            for mt in range(2):
                                ps = psum1.tile([128, ssz], F32, name="pconv",
                                                tag="pconv",
                                                padded_shape=[128, 512])
                                for k in range(8):
                                    nc.tensor.matmul(
                                        ps[:], w1t[k][:, mt * 128:(mt + 1) * 128],
                                        xts[k][:, so:so + ssz],
                                        start=(k == 0), stop=(k == 7))
                                idx = mt * 6 + cstat
                                nc.scalar.activation(Y1[mt][:, off:off + ssz],
                                                     ps[:], AF.Copy,
                                                     accum_out=S1[:, idx:idx + 1])
                                sq = sq1pool.tile([128, ssz], F32, name="sqs",
                                                  tag="sqs",
                                                  padded_shape=[128, 512])
                                nc.scalar.activation(sq[:], ps[:], AF.Square,
                                                     accum_out=Q1[:, idx:idx + 1])
                            cstat += 1

                # ---- phase 1b: BN1 AllReduce + coeffs (DVE); PE keeps warm ----
                s1sum = wpool.tile([128, 2], F32, name="s1sum")
                q1sum = wpool.tile([128, 2], F32, name="q1sum")
                nc.vector.reduce_sum(s1sum[:], S1[:].rearrange("p (m c) -> p m c", c=6), X_AXIS)
                nc.vector.reduce_sum(q1sum[:], Q1[:].rearrange("p (m c) -> p m c", c=6), X_AXIS)
                st1 = wpool.tile([128, 4], F32, name="st1")
                nc.vector.tensor_copy(st1[:, 0:2], s1sum[:])
                nc.vector.tensor_copy(st1[:, 2:4], q1sum[:])
                tot1 = wpool.tile([128, 4], F32, name="tot1")
                stats_allreduce(st1, 4, tot1, "bn1")
                s1c = wpool.tile([128, 2], F32, name="s1c")
                c1c = wpool.tile([128, 2], F32, name="c1c")
                bn_coeffs(tot1, g1t, b1t, CNT12, 2, s1c, c1c, "bn1")

                # ---- phase 2: out1 = relu(s*y1+c) in place, 3-engine split ----
                OUT1 = [Y1[m][:] for m in range(2)]
                for mt in range(2):
                    for cg, (off, sz) in enumerate(CGX):
                        ap = Y1[mt][:, off:off + sz]
                        if cg < 4:
                            nc.scalar.activation(ap, ap, AF.Relu,
                                                 bias=c1c[:, mt:mt + 1],
                                                 scale=s1c[:, mt:mt + 1])
                        else:
                            nc.vector.tensor_scalar(ap, ap,
                                                    s1c[:, mt:mt + 1],
                                                    c1c[:, mt:mt + 1],
                                                    AX.mult, AX.add)
                            nc.vector.tensor_scalar(ap, ap, 0.0, None, AX.max)

                # ---- phase 3a: vT = out1^T @ WvT (+bv), into VTON (ScalarE) ----
                with tc.tile_pool(name="psum3", bufs=1, space="PSUM") as psum3:
                    # keep the PE clock warm through the BN1 collective window
                    warm_chain(psum3, "warm1", [128, 512])
                    for t, (mo, msz) in enumerate(MT22):
                        ps = psum3.tile([128, C], F32, name="pvt", tag="pvt", bufs=2)
                        nc.tensor.matmul(ps[0:msz, :],
                                         onespad[0:1, 0:msz].bitcast(F32R),
                                         bvrow[:], start=True, stop=False)
                        for k in range(2):
                            nc.tensor.matmul(ps[0:msz, :], OUT1[k][:, mo:mo + msz],
                                             wvt[k][:], start=False, stop=(k == 1))
                        # one strided copy for all 4 heads: [msz, 4, 64]
                        dst = VTON[0:msz, :].rearrange(
                            "p (h c) -> p h c", h=HEADS)[:, :, t * 65:t * 65 + DH]
                        src = ps[0:msz, :].rearrange("p (h c) -> p h c", h=HEADS)
                        nc.scalar.activation(dst, src, AF.Copy)
                        dstb = VTONB[0:msz, :].rearrange(
                            "p (h c) -> p h c", h=HEADS)[:, :, t * 65:t * 65 + DH]
                        nc.vector.tensor_copy(dstb, src)

                    # ---- phase 3b: KHAT = [k;q] packed, QHAT q-half ----
                    # bias adds split Scalar (Identity+bias) / DVE; GpSimd
                    # cannot read PSUM.
                    for h in range(HEADS):
                        hs = h * DH
                        for ci, (off, sz) in enumerate(CH6):
                            ps = psum3.tile([128, sz], F32, name="pkh", tag="pkh",
                                            bufs=3)
                            for k in range(2):
                                nc.tensor.matmul(ps[:], wkqt[h][k][:],
                                                 OUT1[k][:, off:off + sz],
                                                 start=(k == 0), stop=(k == 1))
                            if ci % 2 == 0:
                                nc.scalar.activation(KHAT[h][:, off:off + sz],
                                                     ps[:], AF.Identity,
                                                     bias=bkqt[:, h:h + 1])
                            else:
                                nc.vector.tensor_scalar(KHAT[h][:, off:off + sz],
                                                        ps[:], bkqt[:, h:h + 1],
                                                        None, AX.add)
                        for (off, sz) in CH3:
                            pq = psum3.tile([DH, sz], F32, name="pqh", tag="pqh",
                                            bufs=2)
                            for k in range(2):
                                nc.tensor.matmul(pq[:], wqt[k][:, hs:hs + DH],
                                                 OUT1[k][:, off:off + sz],
                                                 start=(k == 0), stop=(k == 1))
                            nc.vector.tensor_scalar(QHAT[h][0:DH, off:off + sz], pq[:],
                                                    bqt[:, h:h + 1], None, AX.add)

            # ---- phase 4: attention ----
            # queries 0:1024 per head run as a Scalar-dense 1024-wide loop;
            # the 348-query tails of two heads are interleaved afterwards so
            # their serial logits->exp->AV chains overlap.  The softmax
            # denominator reciprocal is exp(-ln(d)) on ScalarE (a [1,512]
            # DVE reciprocal costs 3us; ScalarE does the pair in 1.1us).
            with tc.tile_pool(name="oattp", bufs=1) as oattp, \
                 tc.tile_pool(name="epool", bufs=1) as epool:
                OATT = [oattp.tile([DH, NQ], F32R, name=f"oatt{h}")
                        for h in range(HEADS)]

                def post_group(pool, pav, h, off, sz, idx, pbtag, pbbufs):
                    rcr = epool.tile([1, sz], F32, name="rcr", tag="rcr",
                                     bufs=2, padded_shape=[1, 512])
                    nc.vector.reciprocal(rcr[:], pav[DH:65, :])
                    pb = pool.tile([DH, sz], F32, name="pb", tag=pbtag,
                                   bufs=pbbufs, padded_shape=[DH, 512])
                    nc.tensor.matmul(pb[:], onespad[0:1, 0:DH], rcr[:],
                                     start=True, stop=True)
                    pbs = epool.tile([DH, sz], F32, name="pbs", tag="pbs",
                                     bufs=2, padded_shape=[DH, 512])
                    nc.vector.tensor_copy(pbs[:], pb[:])
                    nc.vector.scalar_tensor_tensor(
                        OATT[h][:, off:off + sz], pav[0:DH, :], 1.0, pbs[:],
                        AX.mult, AX.mult, accum_out=S2[:, idx:idx + 1])
                    sq2 = epool.tile([DH, sz], F32, name="sq2", tag="sq2",
                                     bufs=1, padded_shape=[DH, 512])
                    nc.vector.scalar_tensor_tensor(
                        sq2[:], OATT[h][:, off:off + sz], 1.0,
                        OATT[h][:, off:off + sz],
                        AX.mult, AX.mult, accum_out=Q2[:, idx:idx + 1])

                with tc.tile_pool(name="psum4a", bufs=1, space="PSUM") as ps4a:
                    for h in range(HEADS):
                        pavs = [ps4a.tile([65, 512], F32, name=f"pav{si}",
                                          tag=f"pav{si}", bufs=1)
                                for si in range(2)]
                        for t, (mo, msz) in enumerate(MT22):
                            ps = ps4a.tile([128, 1024], F32, name="ps", tag="ps",
                                           bufs=2)
                            for so in (0, 512):
                                nc.tensor.matmul(ps[0:msz, so:so + 512],
                                                 KHAT[h][:, mo:mo + msz],
                                                 QHAT[h][:, so:so + 512],
                                                 start=True, stop=True)
                            e = epool.tile([128, 1024], F32R, name="e", tag="e",
                                           bufs=3)
                            nc.scalar.activation(e[0:msz, :], ps[0:msz, :],
                                                 AF.Exp)
                            base = (h * 22 + t) * 65
                            for si, so in enumerate((0, 512)):
                                nc.tensor.matmul(pavs[si][:],
                                                 VTON[0:msz, base:base + 65],
                                                 e[0:msz, so:so + 512],
                                                 start=(t == 0), stop=(t == 21))
                        for si in range(2):
                            post_group(ps4a, pavs[si], h, si * 512, 512,
                                       h * 3 + si, "pb", 1)

                with tc.tile_pool(name="psum4b", bufs=1, space="PSUM") as ps4b:
                    for pair in ((0, 1), (2, 3)):
                        pav2 = {h: ps4b.tile([65, 348], F32, name=f"pav2_{h}",
                                             tag=f"pav2_{h % 2}", bufs=1,
                                             padded_shape=[65, 512])
                                for h in pair}
                        for t, (mo, msz) in enumerate(MT22):
                            for h in pair:
                                ps = ps4b.tile([128, 348], F32, name="ps2",
                                               tag="ps2", bufs=4,
                                               padded_shape=[128, 512])
                                nc.tensor.matmul(ps[0:msz, :],
                                                 KHAT[h][:, mo:mo + msz],
                                                 QHAT[h][:, 1024:1372],
                                                 start=True, stop=True)
                                base = (h * 22 + t) * 65
                                if h % 2 == 0:
                                    e2 = epool.tile([128, 348], F32R, name="e2f",
                                                    tag="e2f", bufs=2,
                                                    padded_shape=[128, 512])
                                    nc.scalar.activation(e2[0:msz, :],
                                                         ps[0:msz, :], AF.Exp)
                                    vt = VTON
                                else:
                                    e2 = epool.tile([128, 348], BF16, name="e2b",
                                                    tag="e2b", bufs=2,
                                                    padded_shape=[128, 512])
                                    nc.vector.tensor_scalar(
                                        e2[0:msz, :].bitcast(I16), ps[0:msz, :],
                                        SCHR_A16, SCHR_B16, AX.mult, AX.add)
                                    vt = VTONB
                                nc.tensor.matmul(pav2[h][:],
                                                 vt[0:msz, base:base + 65],
                                                 e2[0:msz, :],
                                                 start=(t == 0), stop=(t == 21))
                        for h in pair:
                            post_group(ps4b, pav2[h], h, 1024, 348,
                                       h * 3 + 2, "pb2", 2)
                            # move raw head output into OUT2 layout
                            nc.sync.dma_start(
                                OUT2[h // 2][(h % 2) * DH:(h % 2) * DH + DH, :],
                                OATT[h][:])

                # ---- phase 5: BN2 AllReduce + fused Relu apply ----
                s2sum = wpool.tile([DH, 4], F32, name="s2sum")
                q2sum = wpool.tile([DH, 4], F32, name="q2sum")
                nc.vector.reduce_sum(s2sum[:], S2[:].rearrange("p (h c) -> p h c", c=3), X_AXIS)
                nc.vector.reduce_sum(q2sum[:], Q2[:].rearrange("p (h c) -> p h c", c=3), X_AXIS)
                st2 = wpool.tile([DH, 8], F32, name="st2")
                nc.vector.tensor_copy(st2[:, 0:4], s2sum[:])
                nc.vector.tensor_copy(st2[:, 4:8], q2sum[:])
                tot2 = wpool.tile([DH, 8], F32, name="tot2")
                stats_allreduce(st2, 8, tot2, "bn2")
                s2c = wpool.tile([DH, 4], F32, name="s2c")
                c2c = wpool.tile([DH, 4], F32, name="c2c")
                bn_coeffs(tot2, g2t, b2t, CNT2, 4, s2c, c2c, "bn2")
                # rearrange [64,4] head coeffs -> [128,2] OUT2 channel layout
                s2c128 = wpool.tile([128, 2], F32, name="s2c128")
                c2c128 = wpool.tile([128, 2], F32, name="c2c128")
                for s in range(2):
                    nc.sync.dma_start(
                        s2c128[s * DH:(s + 1) * DH, :],
                        s2c[:].rearrange("d (m s) -> d m s", s=2)[:, :, s])
                    nc.sync.dma_start(
                        c2c128[s * DH:(s + 1) * DH, :],
                        c2c[:].rearrange("d (m s) -> d m s", s=2)[:, :, s])
                for m in range(2):
                    nc.scalar.activation(OUT2[m][:], OUT2[m][:], AF.Relu,
                                         bias=c2c128[:, m:m + 1],
                                         scale=s2c128[:, m:m + 1])

        # ---- phase 6: y3 = W3 @ out2; stats Scalar+DVE; finale 3-engine ----
        with tc.tile_pool(name="y3pool", bufs=1) as y3pool, \
             tc.tile_pool(name="fpool", bufs=2) as fpool, \
             tc.tile_pool(name="psum6", bufs=3, space="PSUM") as psum6:
            # keep the PE clock warm through the BN2 collective window
            warm_chain(psum6, "warm2", [128, 512], bufs=1)
            Y3 = [y3pool.tile([128, NQ], BF16, name=f"y3_{m}") for m in range(8)]
            for mt in range(8):
                for ci, (off, sz) in enumerate(CH3):
                    ps = psum6.tile([128, sz], F32, name="pw3", tag="pw3")
                    for k in range(2):
                        nc.tensor.matmul(ps[:], w3t[k][:, mt * 128:(mt + 1) * 128],
                                         OUT2[k][:, off:off + sz],
                                         start=(k == 0), stop=(k == 1))
                    idx = mt * 3 + ci
                    nc.scalar.activation(Y3[mt][:, off:off + sz], ps[:], AF.Copy,
                                         accum_out=S3[:, idx:idx + 1])
                    sq = fpool.tile([128, sz], F32, name="sq3", tag="sq3")
                    nc.vector.scalar_tensor_tensor(
                        sq[:], Y3[mt][:, off:off + sz], 1.0,
                        Y3[mt][:, off:off + sz], AX.mult, AX.mult,
                        accum_out=Q3[:, idx:idx + 1])

            s3sum = wpool.tile([128, 8], F32, name="s3sum")
            q3sum = wpool.tile([128, 8], F32, name="q3sum")
            nc.vector.reduce_sum(s3sum[:], S3[:].rearrange("p (m c) -> p m c", c=3), X_AXIS)
            nc.vector.reduce_sum(q3sum[:], Q3[:].rearrange("p (m c) -> p m c", c=3), X_AXIS)
            st3 = wpool.tile([128, 16], F32, name="st3")
            nc.vector.tensor_copy(st3[:, 0:8], s3sum[:])
            nc.vector.tensor_copy(st3[:, 8:16], q3sum[:])
            tot3 = wpool.tile([128, 16], F32, name="tot3")
            stats_allreduce(st3, 16, tot3, "bn3")
            s3c = wpool.tile([128, 8], F32, name="s3c")
            c3c = wpool.tile([128, 8], F32, name="c3c")
            bn_coeffs(tot3, g3t, b3t, CNT2, 8, s3c, c3c, "bn3")

            # finale: out = relu(s3*y3 + c3 + x); op1 on DVE (stt with the
            # resident bf16 X tiles), op2 split Scalar (cg 0,2) / DVE (cg 1)
            for mt in range(8):
                for cg in range(3):
                    off, sz = CGX[cg]
                    tf = fpool.tile([128, sz], BF16, name="tf", tag="tf", bufs=3,
                                    padded_shape=[128, 512])
                    nc.vector.scalar_tensor_tensor(
                        tf[:], Y3[mt][:, off:off + sz], s3c[:, mt:mt + 1],
                        XRES[cg][mt], AX.mult, AX.add)
                    to = fpool.tile([128, sz], F32, name="to", tag="to", bufs=3,
                                    padded_shape=[128, 512])
                    nc.scalar.activation(to[:], tf[:], AF.Relu,
                                         bias=c3c[:, mt:mt + 1])
                    oo = OOFF[(mt, cg)]
                    nc.sync.dma_start(OUTd[oo:oo + 128 * sz].rearrange(
                        "(p f) -> p f", p=128), to[:])


_NC_CACHE = {}


def _get_program():
    if "nc" not in _NC_CACHE:
        _NC_CACHE["nc"] = build_program()
    return _NC_CACHE["nc"]


def _host_prep(inputs):
    x = np.ascontiguousarray(inputs["x"].reshape(B, CIN, N))
    rel = (inputs["rel_h"] + inputs["rel_w"] + inputs["rel_d"]).reshape(HEADS, DH, N)
    rel = np.ascontiguousarray(rel.astype(np.float32))
    W1T = np.ascontiguousarray(inputs["W1"].T.astype(ml_dtypes.bfloat16))
    WQT = np.ascontiguousarray(inputs["Wq"].T.astype(np.float32))
    WKT = np.ascontiguousarray(inputs["Wk"].T.astype(np.float32))
    WVT = np.ascontiguousarray(inputs["Wv"].T.astype(np.float32))
    W3T = np.ascontiguousarray(inputs["W3"].T.astype(np.float32))
    WKQ = np.stack([np.concatenate([WKT[:, h * DH:(h + 1) * DH],
                                    WQT[:, h * DH:(h + 1) * DH]], axis=1)
                    for h in range(HEADS)]).astype(np.float32)
    bq, bk, bv = inputs["bq"], inputs["bk"], inputs["bv"]
    BKQ = np.stack([np.concatenate([bk[h * DH:(h + 1) * DH], bq[h * DH:(h + 1) * DH]])
                    for h in range(HEADS)]).astype(np.float32)
    BQ = bq.reshape(HEADS, DH).astype(np.float32)
    BVR = bv.reshape(1, C).astype(np.float32)
    GB1 = np.stack([inputs["g1"], inputs["b1"]]).astype(np.float32)
    GB2 = np.stack([inputs["g2"], inputs["b2"]]).astype(np.float32)
    GB3 = np.stack([inputs["g3"], inputs["b3"]]).astype(np.float32)

    in_maps = []
    for c in range(N_CORES):
        b, s = c // 2, c % 2
        xb = np.roll(x[b], -s * NQ, axis=1).astype(ml_dtypes.bfloat16)
        # tiled layout: contiguous [128, sz] blocks (1 DMA descriptor each)
        xt = np.empty(CIN * N, ml_dtypes.bfloat16)
        for g, (gbase, gw, _subs) in enumerate(XGRP):
            for k in range(8):
                o = XOFF[(g, k)]
                xt[o:o + 128 * gw] = xb[k * 128:(k + 1) * 128,
                                        gbase:gbase + gw].reshape(-1)
        relc = np.ascontiguousarray(rel[:, :, s * NQ:(s + 1) * NQ])
        in_maps.append({
            "X": xt, "W1T": W1T, "WQT": WQT,
            "WVT": WVT, "W3T": W3T, "WKQ": WKQ, "REL": relc, "BKQ": BKQ,
            "BQ": BQ, "BVR": BVR, "GB1": GB1, "GB2": GB2, "GB3": GB3,
        })
    return in_maps


def run(inputs, trace=False, trace_kwargs=None):
    from concourse import bass_utils
    nc = _get_program()
    in_maps = _host_prep(inputs)
    res = bass_utils.run_bass_kernel_spmd(
        nc, in_maps, core_ids=list(range(N_CORES)), trace=trace,
        **(trace_kwargs or {}))
    out = np.empty((B, CIN, N), np.float32)
    for c in range(N_CORES):
        b, s = c // 2, c % 2
        flat = np.asarray(res.results[c]["OUT"]).reshape(-1)
        oc = np.empty((CIN, NQ), np.float32)
        for mt in range(8):
            for cg in range(3):
                off, sz = CGX[cg]
                o = OOFF[(mt, cg)]
                oc[mt * 128:(mt + 1) * 128, off:off + sz] = \
                    flat[o:o + 128 * sz].reshape(128, sz)
        out[b, :, s * NQ:(s + 1) * NQ] = oc
    return out.reshape(B, CIN, 14, 14, 14), res


def kernel(**inputs):
    out, _ = run(inputs, trace=False)
    return out


# revision 36
# speedup vs baseline: 1.1494x; 1.0122x over previous
"""Bottleneck-MHSA fused kernel for 8 Trainium2 NeuronCores (v2).

Sharding: core c = 2*b + s handles batch b; attention queries are split in
half between the two cores of a pair. Each core computes conv1 + BN1 + qkv
for its whole batch (redundantly with its pair partner), then attention for
all 4 heads over its query half, then BN2 + W3 + BN3 + residual for its
query half. Cross-core traffic is only three tiny BN-statistics AllReduces.

v2 changes vs v1 (531us):
- conv1 runs in bf16 (X + W1 host-cast) halving the X DMA that bounded
  phase 1; the residual reuses the resident bf16 X tiles (no re-load).
- BN sum/sumsq stats ride ScalarE activation accum_out (Copy / Square)
  instead of DVE reduce chains; BN applies are single fused
  Relu(scale,bias) activations split across Scalar/DVE/GpSimd.
- Softmax exp is split per query-group: ScalarE does real Exp on ~37%,
  GpSimd+DVE compute Schraudolph exp (one tensor_scalar into an int32
  bitcast) on the rest.  rel-err budget measured on CPU: ~8e-3 << 2e-2.
- Attention post-processing (1/denom broadcast-mult + BN2 stats) moved to
  GpSimd/DVE with double-buffered PSUM accumulators so head/group
  transitions no longer stall the PE (HAM stays warm).
- AllGather+local-reduce -> AllReduce; sqrt via DVE Newton rsqrt so the
  ScalarE activation table never leaves Exp.
- Dummy accumulate-matmul chains keep the PE clock at 2.4GHz through the
  BN1/BN2 collective windows.
"""
import numpy as np
import ml_dtypes

HEADS = 4
DH = 64
C = 256          # PLANES
CIN = 1024       # IN_PLANES
N = 2744         # tokens per batch
NQ = N // 2      # query half per core
B = 4
EPS = 1e-5
N_CORES = 8
CNT12 = 8 * N    # BN1 effective count (pairs double-count; mean/var exact)
CNT2 = 8 * NQ    # BN2/BN3 count (distinct shards)

SCHR_A = 12102203.161561485   # 2^23 / ln 2
SCHR_B = 1064986823.0         # 127*2^23 - 60801*8 (Schraudolph bias)
SCHR_A16 = SCHR_A / 65536.0   # bf16 = top 16 bits of f32
SCHR_B16 = SCHR_B / 65536.0


def _chunks(total, n):
    # even sizes (fp32r matmul requires an even moving free dim)
    assert total % 2 == 0
    half = total // 2
    sizes = [2 * (half // n + (1 if i < half % n else 0)) for i in range(n)]
    out, off = [], 0
    for s in sizes:
        out.append((off, s))
        off += s
    return out


# conv col groups (bf16 X tiles); first 3 cover the residual half 0:1372
CGX = [(0, 512), (512, 512), (1024, 348), (1372, 512), (1884, 512), (2396, 348)]
CH6 = _chunks(N, 6)     # key/token chunks for qkv (456/458 wide)
CH3 = _chunks(NQ, 3)    # query chunks for qkv / W3
MT22 = [(t * 128, min(128, N - t * 128)) for t in range((N + 127) // 128)]
WARM_K = 44             # dummy matmuls per keep-warm chain (~9.5us)

# X DMA groups: (col offset, width, conv sub-splits); first two resident
XGRP = [(0, 1024, [(0, 512), (512, 512)]), (1024, 348, [(0, 348)]),
        (1372, 1024, [(0, 512), (512, 512)]), (2396, 348, [(0, 348)])]
# tiled DRAM offsets: X is staged host-side as contiguous [128, w] blocks
XOFF = {}
_o = 0
for _g, (_off, _w, _) in enumerate(XGRP):
    for _k in range(8):
        XOFF[(_g, _k)] = _o
        _o += 128 * _w
OOFF = {}
_o = 0
for _mt in range(8):
    for _cg in range(3):
        OOFF[(_mt, _cg)] = _o
        _o += 128 * CGX[_cg][1]


def build_program():
    from concourse import bacc, mybir, tile

    F32 = mybir.dt.float32
    F32R = mybir.dt.float32r
    BF16 = mybir.dt.bfloat16
    I32 = mybir.dt.int32

    nc = bacc.Bacc("TRN2", target_bir_lowering=False, debug=False,
                   num_devices=N_CORES)

    # ---- I/O ----
    io = {}
    io["X"] = nc.dram_tensor("X", [CIN * N], BF16, kind="ExternalInput").ap()
    io["W1T"] = nc.dram_tensor("W1T", [CIN, C], BF16, kind="ExternalInput").ap()
    io["WQT"] = nc.dram_tensor("WQT", [C, C], F32R, kind="ExternalInput").ap()
    io["WVT"] = nc.dram_tensor("WVT", [C, C], F32R, kind="ExternalInput").ap()
    io["W3T"] = nc.dram_tensor("W3T", [C, CIN], F32R, kind="ExternalInput").ap()
    io["WKQ"] = nc.dram_tensor("WKQ", [HEADS, C, 128], F32R, kind="ExternalInput").ap()
    io["REL"] = nc.dram_tensor("REL", [HEADS, DH, NQ], F32R, kind="ExternalInput").ap()
    io["BKQ"] = nc.dram_tensor("BKQ", [HEADS, 128], F32, kind="ExternalInput").ap()
    io["BQ"] = nc.dram_tensor("BQ", [HEADS, DH], F32, kind="ExternalInput").ap()
    io["BVR"] = nc.dram_tensor("BVR", [1, C], F32R, kind="ExternalInput").ap()
    io["GB1"] = nc.dram_tensor("GB1", [2, C], F32, kind="ExternalInput").ap()
    io["GB2"] = nc.dram_tensor("GB2", [2, C], F32, kind="ExternalInput").ap()
    io["GB3"] = nc.dram_tensor("GB3", [2, CIN], F32, kind="ExternalInput").ap()
    io["OUT"] = nc.dram_tensor("OUT", [CIN * NQ], F32, kind="ExternalOutput").ap()

    with tile.TileContext(nc) as tc:
        _emit(nc, tc, mybir, F32, F32R, BF16, I32, io)

    nc.compile()
    from concourse.bass_interp import get_hw_module
    nc.m = get_hw_module(nc.m)
    return nc


def _emit(nc, tc, mybir, F32, F32R, BF16, I32, io):
    I16 = mybir.dt.int16
    import contextlib

    AX = mybir.AluOpType
    AF = mybir.ActivationFunctionType
    X_AXIS = mybir.AxisListType.X

    Xd, W1T, WQT, WVT, W3T = io["X"], io["W1T"], io["WQT"], io["WVT"], io["W3T"]
    WKQ, RELd, BKQ, BQd, BVR = io["WKQ"], io["REL"], io["BKQ"], io["BQ"], io["BVR"]
    GB1, GB2, GB3, OUTd = io["GB1"], io["GB2"], io["GB3"], io["OUT"]

    def stats_allreduce(src_sbuf, width, out_sbuf, tag):
        """Sum [P, width] stats over all 8 cores into out_sbuf (AllGather +
        local reduce; measured 2x faster than the AllReduce collective for
        these tiny payloads)."""
        p = src_sbuf.shape[0]
        cin = dpool.tile([p, width], F32, name=f"arin_{tag}")
        cout = dpool.tile([N_CORES, p, width], F32, addr_space="Shared",
                          name=f"arout_{tag}")
        nc.sync.dma_start(cin[:], src_sbuf[:])
        nc.gpsimd.collective_compute(
            "AllGather", AX.bypass,
            replica_groups=[list(range(N_CORES))],
            ins=[cin.opt()], outs=[cout.opt()],
        )
        gath = wpool.tile([p, N_CORES, width], F32, name=f"gath_{tag}")
        nc.sync.dma_start(gath[:], cout[:].rearrange("g p c -> p g c"))
        nc.vector.reduce_sum(out_sbuf[:],
                             gath[:].rearrange("p g c -> p c g"), X_AXIS)

    def rsqrt_newton(y, x, tag):
        """y = 1/sqrt(x) on DVE only (bit-trick seed + 2 Newton steps)."""
        p, w = x.shape[0], x.shape[1]
        xi = x[:].bitcast(I32)
        t1 = wpool.tile([p, w], I32, name=f"rsq_t1_{tag}")
        nc.vector.tensor_scalar(t1[:], xi, 1, None, AX.arith_shift_right)
        yi = y[:].bitcast(I32)
        nc.vector.tensor_scalar(yi, t1[:], -1, 0x5f3759df, AX.mult, AX.add)
        h = wpool.tile([p, w], F32, name=f"rsq_h_{tag}")
        for _ in range(2):
            nc.vector.tensor_tensor(h[:], y[:], y[:], AX.mult)
            nc.vector.tensor_tensor(h[:], x[:], h[:], AX.mult)
            nc.vector.tensor_scalar(h[:], h[:], -0.5, 1.5, AX.mult, AX.add)
            nc.vector.tensor_tensor(y[:], y[:], h[:], AX.mult)

    def bn_coeffs(tot, gt, bt, cnt, w, sc, cc, tag):
        """tot [P, 2w] = [sums | sumsqs] -> scale sc [P, w], bias cc [P, w].
        All DVE (Newton rsqrt) so ScalarE keeps its Exp table loaded."""
        p = tot.shape[0]
        mean = wpool.tile([p, w], F32, name=f"mean_{tag}")
        var = wpool.tile([p, w], F32, name=f"var_{tag}")
        nc.vector.tensor_scalar_mul(mean[:], tot[:, 0:w], 1.0 / cnt)
        nc.vector.tensor_scalar_mul(var[:], tot[:, w:2 * w], 1.0 / cnt)
        m2 = wpool.tile([p, w], F32, name=f"m2_{tag}")
        nc.vector.tensor_tensor(m2[:], mean[:], mean[:], AX.mult)
        nc.vector.tensor_tensor(var[:], var[:], m2[:], AX.subtract)
        nc.vector.tensor_scalar_add(var[:], var[:], EPS)
        rstd = wpool.tile([p, w], F32, name=f"rstd_{tag}")
        rsqrt_newton(rstd, var, tag)
        nc.vector.tensor_tensor(sc[:], gt[:], rstd[:], AX.mult)
        tmp = wpool.tile([p, w], F32, name=f"tmpc_{tag}")
        nc.vector.tensor_tensor(tmp[:], sc[:], mean[:], AX.mult)
        nc.vector.tensor_tensor(cc[:], bt[:], tmp[:], AX.subtract)

    with contextlib.ExitStack() as top:
        wpool = top.enter_context(tc.tile_pool(name="wpool", bufs=1))
        dpool = top.enter_context(tc.tile_pool(name="dpool", bufs=1, space="DRAM"))

        # ---- weights / constants ----
        w1t = []
        for k in range(8):
            t = wpool.tile([128, C], BF16, name=f"w1t{k}")
            nc.scalar.dma_start(t[:], W1T[k * 128:(k + 1) * 128, :])
            w1t.append(t)
        wqt, wvt = [], []
        for srcw, dst, nm in ((WQT, wqt, "wq"), (WVT, wvt, "wv")):
            for k in range(2):
                t = wpool.tile([128, C], F32R, name=f"{nm}{k}")
                nc.scalar.dma_start(t[:], srcw[k * 128:(k + 1) * 128, :])
                dst.append(t)
        wkqt = []
        for h in range(HEADS):
            row = []
            for k in range(2):
                t = wpool.tile([128, 128], F32R, name=f"wkq{h}_{k}")
                nc.scalar.dma_start(t[:], WKQ[h][k * 128:(k + 1) * 128, :])
                row.append(t)
            wkqt.append(row)
        w3t = []
        for k in range(2):
            t = wpool.tile([128, CIN], F32R, name=f"w3t{k}")
            nc.scalar.dma_start(t[:], W3T[k * 128:(k + 1) * 128, :])
            w3t.append(t)
        bvrow = wpool.tile([1, C], F32R, name="bvrow")
        nc.scalar.dma_start(bvrow[:], BVR[:])

        bkqt = wpool.tile([128, HEADS], F32, name="bkqt")
        nc.scalar.dma_start(bkqt[:], BKQ[:].rearrange("h p -> p h"))
        bqt = wpool.tile([DH, HEADS], F32, name="bqt")
        nc.scalar.dma_start(bqt[:], BQd[:].rearrange("h d -> d h"))
        g1t = wpool.tile([128, 2], F32, name="g1t")
        b1t = wpool.tile([128, 2], F32, name="b1t")
        nc.scalar.dma_start(g1t[:], GB1[0].rearrange("(m p) -> p m", p=128))
        nc.scalar.dma_start(b1t[:], GB1[1].rearrange("(m p) -> p m", p=128))
        g2t = wpool.tile([DH, HEADS], F32, name="g2t")
        b2t = wpool.tile([DH, HEADS], F32, name="b2t")
        nc.scalar.dma_start(g2t[:], GB2[0].rearrange("(h d) -> d h", d=DH))
        nc.scalar.dma_start(b2t[:], GB2[1].rearrange("(h d) -> d h", d=DH))
        g3t = wpool.tile([128, 8], F32, name="g3t")
        b3t = wpool.tile([128, 8], F32, name="b3t")
        nc.scalar.dma_start(g3t[:], GB3[0].rearrange("(m p) -> p m", p=128))
        nc.scalar.dma_start(b3t[:], GB3[1].rearrange("(m p) -> p m", p=128))

        # constants: ones pad (bitcast to f32r where needed), warm-chain srcs
        onespad = wpool.tile([128, 128], F32, name="onespad")
        nc.vector.memset(onespad[:], 1.0)
        warmx = wpool.tile([128, 512], F32, name="warmx")
        nc.vector.memset(warmx[:], 0.0)

        def warm_chain(pool, tag, shape, bufs=1, k=WARM_K):
            """Dummy accumulate-matmul chain: keeps the PE clock warm during
            a collective window.  Allocates from the given live PSUM pool."""
            wps = pool.tile(shape, F32, name=f"warm_{tag}", tag=tag, bufs=bufs)
            for i in range(k):
                nc.tensor.matmul(wps[0:128, 0:512], onespad[:].bitcast(F32R),
                                 warmx[:].bitcast(F32R),
                                 start=(i == 0), stop=(i == k - 1))

        # stats accumulators
        S1 = wpool.tile([128, 12], F32, name="S1")   # conv1 sums   (mt*6+cg)
        Q1 = wpool.tile([128, 12], F32, name="Q1")   # conv1 sumsqs
        S2 = wpool.tile([DH, 12], F32, name="S2")    # attn sums    (h*3+gi)
        Q2 = wpool.tile([DH, 12], F32, name="Q2")
        S3 = wpool.tile([128, 24], F32, name="S3")   # W3 sums      (mt*3+ci)
        Q3 = wpool.tile([128, 24], F32, name="Q3")

        # resident bf16 X tiles covering cols 0:1372 (conv input + residual)
        XAB = [wpool.tile([128, 1024], BF16, name=f"xab{k}") for k in range(8)]
        XC = [wpool.tile([128, 348], BF16, name=f"xc{k}") for k in range(8)]
        # residual slices per finale chunk cg in 0..2
        XRES = [[XAB[k][:, 0:512] for k in range(8)],
                [XAB[k][:, 512:1024] for k in range(8)],
                [XC[k][:] for k in range(8)]]
        OUT2 = [wpool.tile([128, NQ], F32R, name=f"out2_{m}") for m in range(2)]

        with contextlib.ExitStack() as ph_a:
            qpool = ph_a.enter_context(tc.tile_pool(name="qpool", bufs=1))
            KHAT = [qpool.tile([128, N], F32R, name=f"khat{h}") for h in range(HEADS)]
            QHAT = [qpool.tile([128, NQ], F32R, name=f"qhat{h}") for h in range(HEADS)]
            # per-head blocks of 22*65 cols: [v^T (64) | ones] per token tile
            VTON = qpool.tile([128, HEADS * 22 * 65], F32R, name="vton")
            # bf16 shadow for the tail-query AV matmuls (Schraudolph path)
            VTONB = qpool.tile([128, HEADS * 22 * 65], BF16, name="vtonb")
            # ones columns (softmax denominator rows), one strided copy per head
            for vt in (VTON, VTONB):
                for h in range(HEADS):
                    dst = vt[:, h * 1430:(h + 1) * 1430].rearrange(
                        "p (t c) -> p t c", t=22)[:, :, DH:DH + 1]
                    src = onespad[:, 0:22].rearrange("p (t c) -> p t c", t=22)
                    nc.vector.tensor_copy(dst, src)
            for h in range(HEADS):
                nc.scalar.dma_start(QHAT[h][DH:128, :], RELd[h])

            with contextlib.ExitStack() as ph1:
                y1pool = ph1.enter_context(tc.tile_pool(name="y1pool", bufs=1))
                Y1 = [y1pool.tile([128, N], F32R, name=f"y1_{m}") for m in range(2)]
                sq1pool = ph1.enter_context(tc.tile_pool(name="sq1pool", bufs=2))

                # ---- phase 1: conv1 (y1 = W1 @ x) bf16, stats on ScalarE ----
                with tc.tile_pool(name="xbpool", bufs=1) as xbpool, \
                     tc.tile_pool(name="psum1", bufs=4, space="PSUM") as psum1:
                    # warm the PE clock while weights/X stream in
                    warm_chain(psum1, "warm0", [128, 512], k=16)
                    dma_engs = (nc.sync, nc.gpsimd)
                    cstat = 0
                    for g, (gbase, gw, subs) in enumerate(XGRP):
                        if g == 0:
                            xts = XAB
                        elif g == 1:
                            xts = XC
                        else:
                            pfx = "xb" if g == 2 else "xd"
                            xts = [xbpool.tile([128, gw], BF16,
                                               name=f"{pfx}{g}_{k}",
                                               tag=f"{pfx}{k}")
                                   for k in range(8)]
                        for k in range(8):
                            xo = XOFF[(g, k)]
                            eng = dma_engs[0] if g < 2 else dma_engs[1]
                            eng.dma_start(
                                xts[k][:], Xd[xo:xo + 128 * gw].rearrange(
                                    "(p f) -> p f", p=128))
                        for so, ssz in subs:
                            off = gbase + so
                            for mt in range(2):
                                ps = psum1.tile([128, ssz], F32, name="pconv",
                                                tag="pconv",
                                                padded_shape=[128, 512])
                                for k in range(8):
                                    nc.tensor.matmul(
                                        ps[:], w1t[k][:, mt * 128:(mt + 1) * 128],
                                        xts[k][:, so:so + ssz],
                                        start=(k == 0), stop=(k == 7))
                                idx = mt * 6 + cstat
                                nc.scalar.activation(Y1[mt][:, off:off + ssz],
                                                     ps[:], AF.Copy,
                                                     accum_out=S1[:, idx:idx + 1])
                                sq = sq1pool.tile([128, ssz], F32, name="sqs",
                                                  tag="sqs",
                                                  padded_shape=[128, 512])
                                nc.scalar.activation(sq[:], ps[:], AF.Square,
                                                     accum_out=Q1[:, idx:idx + 1])
                            cstat += 1

                # ---- phase 1b: BN1 AllReduce + coeffs (DVE); PE keeps warm ----
                s1sum = wpool.tile([128, 2], F32, name="s1sum")
                q1sum = wpool.tile([128, 2], F32, name="q1sum")
                nc.vector.reduce_sum(s1sum[:], S1[:].rearrange("p (m c) -> p m c", c=6), X_AXIS)
                nc.vector.reduce_sum(q1sum[:], Q1[:].rearrange("p (m c) -> p m c", c=6), X_AXIS)
                st1 = wpool.tile([128, 4], F32, name="st1")
                nc.vector.tensor_copy(st1[:, 0:2], s1sum[:])
                nc.vector.tensor_copy(st1[:, 2:4], q1sum[:])
                tot1 = wpool.tile([128, 4], F32, name="tot1")
                stats_allreduce(st1, 4, tot1, "bn1")
                s1c = wpool.tile([128, 2], F32, name="s1c")
                c1c = wpool.tile([128, 2], F32, name="c1c")
                bn_coeffs(tot1, g1t, b1t, CNT12, 2, s1c, c1c, "bn1")

                # ---- phase 2: out1 = relu(s*y1+c) in place, 3-engine split ----
                OUT1 = [Y1[m][:] for m in range(2)]
                for mt in range(2):
                    for cg, (off, sz) in enumerate(CGX):
                        ap = Y1[mt][:, off:off + sz]
                        if cg < 4:
                            nc.scalar.activation(ap, ap, AF.Relu,
                                                 bias=c1c[:, mt:mt + 1],
                                                 scale=s1c[:, mt:mt + 1])
                        else:
                            nc.vector.tensor_scalar(ap, ap,
                                                    s1c[:, mt:mt + 1],
                                                    c1c[:, mt:mt + 1],
                                                    AX.mult, AX.add)
                            nc.vector.tensor_scalar(ap, ap, 0.0, None, AX.max)

                # ---- phase 3a: vT = out1^T @ WvT (+bv), into VTON (ScalarE) ----
                with tc.tile_pool(name="psum3", bufs=1, space="PSUM") as psum3:
                    # keep the PE clock warm through the BN1 collective window
                    warm_chain(psum3, "warm1", [128, 512], k=70)
                    for t, (mo, msz) in enumerate(MT22):
                        ps = psum3.tile([128, C], F32, name="pvt", tag="pvt", bufs=2)
                        nc.tensor.matmul(ps[0:msz, :],
                                         onespad[0:1, 0:msz].bitcast(F32R),
                                         bvrow[:], start=True, stop=False)
                        for k in range(2):
                            nc.tensor.matmul(ps[0:msz, :], OUT1[k][:, mo:mo + msz],
                                             wvt[k][:], start=False, stop=(k == 1))
                        # one strided copy for all 4 heads: [msz, 4, 64]
                        dst = VTON[0:msz, :].rearrange(
                            "p (h c) -> p h c", h=HEADS)[:, :, t * 65:t * 65 + DH]
                        src = ps[0:msz, :].rearrange("p (h c) -> p h c", h=HEADS)
                        nc.scalar.activation(dst, src, AF.Copy)
                        dstb = VTONB[0:msz, :].rearrange(
                            "p (h c) -> p h c", h=HEADS)[:, :, t * 65:t * 65 + DH]
                        nc.vector.tensor_copy(dstb, src)

                    # ---- phase 3b: KHAT = [k;q] packed, QHAT q-half ----
                    # bias adds split Scalar (Identity+bias) / DVE; GpSimd
                    # cannot read PSUM.
                    for h in range(HEADS):
                        hs = h * DH
                        for ci, (off, sz) in enumerate(CH6):
                            ps = psum3.tile([128, sz], F32, name="pkh", tag="pkh",
                                            bufs=3)
                            for k in range(2):
                                nc.tensor.matmul(ps[:], wkqt[h][k][:],
                                                 OUT1[k][:, off:off + sz],
                                                 start=(k == 0), stop=(k == 1))
                            if ci % 2 == 0:
                                nc.scalar.activation(KHAT[h][:, off:off + sz],
                                                     ps[:], AF.Identity,
                                                     bias=bkqt[:, h:h + 1])
                            else:
                                nc.vector.tensor_scalar(KHAT[h][:, off:off + sz],
                                                        ps[:], bkqt[:, h:h + 1],
                                                        None, AX.add)
                        for (off, sz) in CH3:
                            pq = psum3.tile([DH, sz], F32, name="pqh", tag="pqh",
                                            bufs=2)
                            for k in range(2):
                                nc.tensor.matmul(pq[:], wqt[k][:, hs:hs + DH],
                                                 OUT1[k][:, off:off + sz],
                                                 start=(k == 0), stop=(k == 1))
                            nc.vector.tensor_scalar(QHAT[h][0:DH, off:off + sz], pq[:],
                                                    bqt[:, h:h + 1], None, AX.add)

            # ---- phase 4: attention ----
            # queries 0:1024 per head run as a Scalar-dense 1024-wide loop;
            # the 348-query tails of two heads are interleaved afterwards so
            # their serial logits->exp->AV chains overlap.  The softmax
            # denominator reciprocal is exp(-ln(d)) on ScalarE (a [1,512]
            # DVE reciprocal costs 3us; ScalarE does the pair in 1.1us).
            with tc.tile_pool(name="oattp", bufs=1) as oattp, \
                 tc.tile_pool(name="epool", bufs=1) as epool:
                OATT = [oattp.tile([DH, NQ], F32R, name=f"oatt{h}")
                        for h in range(HEADS)]

                def post_group(pool, pav, h, off, sz, idx, pbtag, pbbufs):
                    rcr = epool.tile([1, sz], F32, name="rcr", tag="rcr",
                                     bufs=2, padded_shape=[1, 512])
                    nc.vector.reciprocal(rcr[:], pav[DH:65, :])
                    pb = pool.tile([DH, sz], F32, name="pb", tag=pbtag,
                                   bufs=pbbufs, padded_shape=[DH, 512])
                    nc.tensor.matmul(pb[:], onespad[0:1, 0:DH], rcr[:],
                                     start=True, stop=True)
                    pbs = epool.tile([DH, sz], F32, name="pbs", tag="pbs",
                                     bufs=2, padded_shape=[DH, 512])
                    nc.vector.tensor_copy(pbs[:], pb[:])
                    nc.vector.scalar_tensor_tensor(
                        OATT[h][:, off:off + sz], pav[0:DH, :], 1.0, pbs[:],
                        AX.mult, AX.mult, accum_out=S2[:, idx:idx + 1])
                    sq2 = epool.tile([DH, sz], F32, name="sq2", tag="sq2",
                                     bufs=1, padded_shape=[DH, 512])
                    nc.vector.scalar_tensor_tensor(
                        sq2[:], OATT[h][:, off:off + sz], 1.0,
                        OATT[h][:, off:off + sz],
                        AX.mult, AX.mult, accum_out=Q2[:, idx:idx + 1])

                with tc.tile_pool(name="psum4a", bufs=1, space="PSUM") as ps4a:
                    for h in range(HEADS):
                        pavs = [ps4a.tile([65, 512], F32, name=f"pav{si}",
                                          tag=f"pav{si}", bufs=1)
                                for si in range(2)]
                        for t, (mo, msz) in enumerate(MT22):
                            ps = ps4a.tile([128, 1024], F32, name="ps", tag="ps",
                                           bufs=2)
                            for so in (0, 512):
                                nc.tensor.matmul(ps[0:msz, so:so + 512],
                                                 KHAT[h][:, mo:mo + msz],
                                                 QHAT[h][:, so:so + 512],
                                                 start=True, stop=True)
                            e = epool.tile([128, 1024], F32R, name="e", tag="e",
                                           bufs=3)
                            nc.scalar.activation(e[0:msz, :], ps[0:msz, :],
                                                 AF.Exp)
                            base = (h * 22 + t) * 65
                            for si, so in enumerate((0, 512)):
                                nc.tensor.matmul(pavs[si][:],
                                                 VTON[0:msz, base:base + 65],
                                                 e[0:msz, so:so + 512],
                                                 start=(t == 0), stop=(t == 21))
                        for si in range(2):
                            post_group(ps4a, pavs[si], h, si * 512, 512,
                                       h * 3 + si, "pb", 1)

                with tc.tile_pool(name="psum4b", bufs=1, space="PSUM") as ps4b:
                    for pair in ((0, 1), (2, 3)):
                        pav2 = {h: ps4b.tile([65, 348], F32, name=f"pav2_{h}",
                                             tag=f"pav2_{h % 2}", bufs=1,
                                             padded_shape=[65, 512])
                                for h in pair}
                        for t, (mo, msz) in enumerate(MT22):
                            for h in pair:
                                ps = ps4b.tile([128, 348], F32, name="ps2",
                                               tag="ps2", bufs=4,
                                               padded_shape=[128, 512])
                                nc.tensor.matmul(ps[0:msz, :],
                                                 KHAT[h][:, mo:mo + msz],
                                                 QHAT[h][:, 1024:1372],
                                                 start=True, stop=True)
                                base = (h * 22 + t) * 65
                                if h % 2 == 0:
                                    e2 = epool.tile([128, 348], F32R, name="e2f",
                                                    tag="e2f", bufs=2,
                                                    padded_shape=[128, 512])
                                    nc.scalar.activation(e2[0:msz, :],
                                                         ps[0:msz, :], AF.Exp)
                                    vt = VTON
                                else:
                                    e2 = epool.tile([128, 348], BF16, name="e2b",
                                                    tag="e2b", bufs=2,
                                                    padded_shape=[128, 512])
                                    nc.vector.tensor_scalar(
                                        e2[0:msz, :].bitcast(I16), ps[0:msz, :],
                                        SCHR_A16, SCHR_B16, AX.mult, AX.add)
                                    vt = VTONB
                                nc.tensor.matmul(pav2[h][:],
                                                 vt[0:msz, base:base + 65],
                                                 e2[0:msz, :],
                                                 start=(t == 0), stop=(t == 21))
                        for h in pair:
                            post_group(ps4b, pav2[h], h, 1024, 348,
                                       h * 3 + 2, "pb2", 2)
                            # move raw head output into OUT2 layout
                            nc.sync.dma_start(
                                OUT2[h // 2][(h % 2) * DH:(h % 2) * DH + DH, :],
                                OATT[h][:])

                # ---- phase 5: BN2 AllReduce + fused Relu apply ----
                s2sum = wpool.tile([DH, 4], F32, name="s2sum")
                q2sum = wpool.tile([DH, 4], F32, name="q2sum")
                nc.vector.reduce_sum(s2sum[:], S2[:].rearrange("p (h c) -> p h c", c=3), X_AXIS)
                nc.vector.reduce_sum(q2sum[:], Q2[:].rearrange("p (h c) -> p h c", c=3), X_AXIS)
                st2 = wpool.tile([DH, 8], F32, name="st2")
                nc.vector.tensor_copy(st2[:, 0:4], s2sum[:])
                nc.vector.tensor_copy(st2[:, 4:8], q2sum[:])
                tot2 = wpool.tile([DH, 8], F32, name="tot2")
                stats_allreduce(st2, 8, tot2, "bn2")
                s2c = wpool.tile([DH, 4], F32, name="s2c")
                c2c = wpool.tile([DH, 4], F32, name="c2c")
                bn_coeffs(tot2, g2t, b2t, CNT2, 4, s2c, c2c, "bn2")
                # rearrange [64,4] head coeffs -> [128,2] OUT2 channel layout
                s2c128 = wpool.tile([128, 2], F32, name="s2c128")
                c2c128 = wpool.tile([128, 2], F32, name="c2c128")
                for s in range(2):
                    nc.sync.dma_start(
                        s2c128[s * DH:(s + 1) * DH, :],
                        s2c[:].rearrange("d (m s) -> d m s", s=2)[:, :, s])
                    nc.sync.dma_start(
                        c2c128[s * DH:(s + 1) * DH, :],
                        c2c[:].rearrange("d (m s) -> d m s", s=2)[:, :, s])
                for m in range(2):
                    nc.scalar.activation(OUT2[m][:], OUT2[m][:], AF.Relu,
                                         bias=c2c128[:, m:m + 1],
                                         scale=s2c128[:, m:m + 1])

        # ---- phase 6: y3 = W3 @ out2; stats Scalar+DVE; finale 3-engine ----
        with tc.tile_pool(name="y3pool", bufs=1) as y3pool, \
             tc.tile_pool(name="fpool", bufs=2) as fpool, \
             tc.tile_pool(name="psum6", bufs=3, space="PSUM") as psum6:
            # keep the PE clock warm through the BN2 collective window
            warm_chain(psum6, "warm2", [128, 512], bufs=1)
            Y3 = [y3pool.tile([128, NQ], BF16, name=f"y3_{m}") for m in range(8)]
            for mt in range(8):
                for ci, (off, sz) in enumerate(CH3):
                    ps = psum6.tile([128, sz], F32, name="pw3", tag="pw3")
                    for k in range(2):
                        nc.tensor.matmul(ps[:], w3t[k][:, mt * 128:(mt + 1) * 128],
                                         OUT2[k][:, off:off + sz],
                                         start=(k == 0), stop=(k == 1))
                    idx = mt * 3 + ci
                    nc.scalar.activation(Y3[mt][:, off:off + sz], ps[:], AF.Copy,
                                         accum_out=S3[:, idx:idx + 1])
                    sq = fpool.tile([128, sz], F32, name="sq3", tag="sq3")
                    nc.vector.scalar_tensor_tensor(
                        sq[:], Y3[mt][:, off:off + sz], 1.0,
                        Y3[mt][:, off:off + sz], AX.mult, AX.mult,
                        accum_out=Q3[:, idx:idx + 1])

            s3sum = wpool.tile([128, 8], F32, name="s3sum")
            q3sum = wpool.tile([128, 8], F32, name="q3sum")
            nc.vector.reduce_sum(s3sum[:], S3[:].rearrange("p (m c) -> p m c", c=3), X_AXIS)
            nc.vector.reduce_sum(q3sum[:], Q3[:].rearrange("p (m c) -> p m c", c=3), X_AXIS)
            st3 = wpool.tile([128, 16], F32, name="st3")
            nc.vector.tensor_copy(st3[:, 0:8], s3sum[:])
            nc.vector.tensor_copy(st3[:, 8:16], q3sum[:])
            tot3 = wpool.tile([128, 16], F32, name="tot3")
            stats_allreduce(st3, 16, tot3, "bn3")
            s3c = wpool.tile([128, 8], F32, name="s3c")
            c3c = wpool.tile([128, 8], F32, name="c3c")
            bn_coeffs(tot3, g3t, b3t, CNT2, 8, s3c, c3c, "bn3")

            # finale: out = relu(s3*y3 + c3 + x); op1 on DVE (stt with the
            # resident bf16 X tiles), op2 split Scalar (cg 0,2) / DVE (cg 1)
            for mt in range(8):
                for cg in range(3):
                    off, sz = CGX[cg]
                    tf = fpool.tile([128, sz], BF16, name="tf", tag="tf", bufs=3,
                                    padded_shape=[128, 512])
                    nc.vector.scalar_tensor_tensor(
                        tf[:], Y3[mt][:, off:off + sz], s3c[:, mt:mt + 1],
                        XRES[cg][mt], AX.mult, AX.add)
                    to = fpool.tile([128, sz], F32, name="to", tag="to", bufs=3,
                                    padded_shape=[128, 512])
                    nc.scalar.activation(to[:], tf[:], AF.Relu,
                                         bias=c3c[:, mt:mt + 1])
                    oo = OOFF[(mt, cg)]
                    nc.sync.dma_start(OUTd[oo:oo + 128 * sz].rearrange(
                        "(p f) -> p f", p=128), to[:])


_NC_CACHE = {}


def _get_program():
    if "nc" not in _NC_CACHE:
        _NC_CACHE["nc"] = build_program()
    return _NC_CACHE["nc"]


def _host_prep(inputs):
    x = np.ascontiguousarray(inputs["x"].reshape(B, CIN, N))
    rel = (inputs["rel_h"] + inputs["rel_w"] + inputs["rel_d"]).reshape(HEADS, DH, N)
    rel = np.ascontiguousarray(rel.astype(np.float32))
    W1T = np.ascontiguousarray(inputs["W1"].T.astype(ml_dtypes.bfloat16))
    WQT = np.ascontiguousarray(inputs["Wq"].T.astype(np.float32))
    WKT = np.ascontiguousarray(inputs["Wk"].T.astype(np.float32))
    WVT = np.ascontiguousarray(inputs["Wv"].T.astype(np.float32))
    W3T = np.ascontiguousarray(inputs["W3"].T.astype(np.float32))
    WKQ = np.stack([np.concatenate([WKT[:, h * DH:(h + 1) * DH],
                                    WQT[:, h * DH:(h + 1) * DH]], axis=1)
                    for h in range(HEADS)]).astype(np.float32)
    bq, bk, bv = inputs["bq"], inputs["bk"], inputs["bv"]
    BKQ = np.stack([np.concatenate([bk[h * DH:(h + 1) * DH], bq[h * DH:(h + 1) * DH]])
                    for h in range(HEADS)]).astype(np.float32)
    BQ = bq.reshape(HEADS, DH).astype(np.float32)
    BVR = bv.reshape(1, C).astype(np.float32)
    GB1 = np.stack([inputs["g1"], inputs["b1"]]).astype(np.float32)
    GB2 = np.stack([inputs["g2"], inputs["b2"]]).astype(np.float32)
    GB3 = np.stack([inputs["g3"], inputs["b3"]]).astype(np.float32)

    in_maps = []
    for c in range(N_CORES):
        b, s = c // 2, c % 2
        xb = np.roll(x[b], -s * NQ, axis=1).astype(ml_dtypes.bfloat16)
        # tiled layout: contiguous [128, sz] blocks (1 DMA descriptor each)
        xt = np.empty(CIN * N, ml_dtypes.bfloat16)
        for g, (gbase, gw, _subs) in enumerate(XGRP):
            for k in range(8):
                o = XOFF[(g, k)]
                xt[o:o + 128 * gw] = xb[k * 128:(k + 1) * 128,
                                        gbase:gbase + gw].reshape(-1)
        relc = np.ascontiguousarray(rel[:, :, s * NQ:(s + 1) * NQ])
        in_maps.append({
            "X": xt, "W1T": W1T, "WQT": WQT,
            "WVT": WVT, "W3T": W3T, "WKQ": WKQ, "REL": relc, "BKQ": BKQ,
            "BQ": BQ, "BVR": BVR, "GB1": GB1, "GB2": GB2, "GB3": GB3,
        })
    return in_maps


def run(inputs, trace=False, trace_kwargs=None):
    from concourse import bass_utils
    nc = _get_program()
    in_maps = _host_prep(inputs)
    res = bass_utils.run_bass_kernel_spmd(
        nc, in_maps, core_ids=list(range(N_CORES)), trace=trace,
        **(trace_kwargs or {}))
    out = np.empty((B, CIN, N), np.float32)
    for c in range(N_CORES):
        b, s = c // 2, c % 2
        flat = np.asarray(res.results[c]["OUT"]).reshape(-1)
        oc = np.empty((CIN, NQ), np.float32)
        for mt in range(8):
            for cg in range(3):
                off, sz = CGX[cg]
                o = OOFF[(mt, cg)]
                oc[mt * 128:(mt + 1) * 128, off:off + sz] = \
                    flat[o:o + 128 * sz].reshape(128, sz)
        out[b, :, s * NQ:(s + 1) * NQ] = oc
    return out.reshape(B, CIN, 14, 14, 14), res


def kernel(**inputs):
    out, _ = run(inputs, trace=False)
    return out


# revision 38
# speedup vs baseline: 1.1623x; 1.0112x over previous
"""Bottleneck-MHSA fused kernel for 8 Trainium2 NeuronCores.

Sharding: core c = 2*b + s handles batch b; attention queries are split in
half between the two cores of a pair. Each core computes conv1 + BN1 + qkv
for its whole batch (redundantly with its pair partner), then attention for
all 4 heads over its query half, then BN2 + W3 + BN3 + residual for its
query half. Cross-core traffic is only three tiny BN-statistics AllGathers.

Optimizations vs the 531us v1 (measured 442us, rel err 6.8e-3):
- conv1 runs in bf16 (X + W1 host-cast), halving the X DMA that bounded
  phase 1; X is staged host-side in a tiled layout so every [128, w] tile
  is one contiguous DMA, and the residual reuses the resident bf16 tiles.
- BN sum/sumsq stats ride ScalarE activation accum_out (Copy / Square);
  BN applies are fused Relu(scale, bias) activations; BN coeffs use a DVE
  Newton rsqrt so the ScalarE table never leaves the exp set.
- The stats AllGather result is fetched without an element transpose
  (the transposed gather cost ~23us in per-element descriptors); the
  cross-core reduce runs on a free-dim-reordered AP instead.
- Attention: queries 0:1024 per head use 1024-wide ScalarE Exp; the 348
  tail queries of two heads at a time are interleaved so their serial
  logits->exp->AV chains overlap, with odd heads' tail exp computed as a
  one-instruction Schraudolph bf16 exp on DVE (feeding a bf16 AV matmul
  against a bf16 VTON shadow) to unload the ScalarE bottleneck.
- Attention epilogue: denominator recip on DVE, partition-broadcast via a
  tiny fp32 ones-matmul, normalize+BN2-row-sums fused in one DVE op;
  raw head outputs DMA into the OUT2 layout during attention.
- OUT is written in a tiled layout (contiguous stores) and un-tiled on
  host; finale computes relu(s3*y3 + c3 + x) with a bf16 intermediate
  (2x DVE mode) and ScalarE relu.
- Dummy accumulate-matmul chains keep the PE clock (HAM) at 2.4GHz
  through the kernel start and the BN1/BN2 collective windows.
"""
import numpy as np
import ml_dtypes

HEADS = 4
DH = 64
C = 256          # PLANES
CIN = 1024       # IN_PLANES
N = 2744         # tokens per batch
NQ = N // 2      # query half per core
B = 4
EPS = 1e-5
N_CORES = 8
CNT12 = 8 * N    # BN1 effective count (pairs double-count; mean/var exact)
CNT2 = 8 * NQ    # BN2/BN3 count (distinct shards)

SCHR_A = 12102203.161561485   # 2^23 / ln 2
SCHR_B = 1064986823.0         # 127*2^23 - 60801*8 (Schraudolph bias)
SCHR_A16 = SCHR_A / 65536.0   # bf16 = top 16 bits of f32
SCHR_B16 = SCHR_B / 65536.0


def _chunks(total, n):
    # even sizes (fp32r matmul requires an even moving free dim)
    assert total % 2 == 0
    half = total // 2
    sizes = [2 * (half // n + (1 if i < half % n else 0)) for i in range(n)]
    out, off = [], 0
    for s in sizes:
        out.append((off, s))
        off += s
    return out


# conv col groups (bf16 X tiles); first 3 cover the residual half 0:1372
CGX = [(0, 512), (512, 512), (1024, 348), (1372, 512), (1884, 512), (2396, 348)]
CH6 = _chunks(N, 6)     # key/token chunks for qkv (456/458 wide)
CH3 = _chunks(NQ, 3)    # query chunks for qkv / W3
MT22 = [(t * 128, min(128, N - t * 128)) for t in range((N + 127) // 128)]
WARM_K = 44             # dummy matmuls per keep-warm chain (~9.5us)

# X DMA groups: (col offset, width, conv sub-splits); first two resident
XGRP = [(0, 1024, [(0, 512), (512, 512)]), (1024, 348, [(0, 348)]),
        (1372, 1024, [(0, 512), (512, 512)]), (2396, 348, [(0, 348)])]
# tiled DRAM offsets: X is staged host-side as contiguous [128, w] blocks
XOFF = {}
_o = 0
for _g, (_off, _w, _) in enumerate(XGRP):
    for _k in range(8):
        XOFF[(_g, _k)] = _o
        _o += 128 * _w
OOFF = {}
_o = 0
for _mt in range(8):
    for _cg in range(3):
        OOFF[(_mt, _cg)] = _o
        _o += 128 * CGX[_cg][1]


def build_program():
    from concourse import bacc, mybir, tile

    F32 = mybir.dt.float32
    F32R = mybir.dt.float32r
    BF16 = mybir.dt.bfloat16
    I32 = mybir.dt.int32

    nc = bacc.Bacc("TRN2", target_bir_lowering=False, debug=False,
                   num_devices=N_CORES)

    # ---- I/O ----
    io = {}
    io["X"] = nc.dram_tensor("X", [CIN * N], BF16, kind="ExternalInput").ap()
    io["W1T"] = nc.dram_tensor("W1T", [CIN, C], BF16, kind="ExternalInput").ap()
    io["WQT"] = nc.dram_tensor("WQT", [C, C], F32R, kind="ExternalInput").ap()
    io["WVT"] = nc.dram_tensor("WVT", [C, C], F32R, kind="ExternalInput").ap()
    io["W3T"] = nc.dram_tensor("W3T", [C, CIN], F32R, kind="ExternalInput").ap()
    io["WKQ"] = nc.dram_tensor("WKQ", [HEADS, C, 128], F32R, kind="ExternalInput").ap()
    io["REL"] = nc.dram_tensor("REL", [HEADS, DH, NQ], F32R, kind="ExternalInput").ap()
    io["BKQ"] = nc.dram_tensor("BKQ", [HEADS, 128], F32, kind="ExternalInput").ap()
    io["BQ"] = nc.dram_tensor("BQ", [HEADS, DH], F32, kind="ExternalInput").ap()
    io["BVR"] = nc.dram_tensor("BVR", [1, C], F32R, kind="ExternalInput").ap()
    io["GB1"] = nc.dram_tensor("GB1", [2, C], F32, kind="ExternalInput").ap()
    io["GB2"] = nc.dram_tensor("GB2", [2, C], F32, kind="ExternalInput").ap()
    io["GB3"] = nc.dram_tensor("GB3", [2, CIN], F32, kind="ExternalInput").ap()
    io["OUT"] = nc.dram_tensor("OUT", [CIN * NQ], F32, kind="ExternalOutput").ap()

    with tile.TileContext(nc) as tc:
        _emit(nc, tc, mybir, F32, F32R, BF16, I32, io)

    nc.compile()
    from concourse.bass_interp import get_hw_module
    nc.m = get_hw_module(nc.m)
    return nc


def _emit(nc, tc, mybir, F32, F32R, BF16, I32, io):
    I16 = mybir.dt.int16
    import contextlib

    AX = mybir.AluOpType
    AF = mybir.ActivationFunctionType
    X_AXIS = mybir.AxisListType.X

    Xd, W1T, WQT, WVT, W3T = io["X"], io["W1T"], io["WQT"], io["WVT"], io["W3T"]
    WKQ, RELd, BKQ, BQd, BVR = io["WKQ"], io["REL"], io["BKQ"], io["BQ"], io["BVR"]
    GB1, GB2, GB3, OUTd = io["GB1"], io["GB2"], io["GB3"], io["OUT"]

    def stats_allreduce(src_sbuf, width, out_sbuf, tag):
        """Sum [P, width] stats over all 8 cores into out_sbuf (AllGather +
        local reduce; measured 2x faster than the AllReduce collective for
        these tiny payloads)."""
        p = src_sbuf.shape[0]
        cin = dpool.tile([p, width], F32, name=f"arin_{tag}")
        cout = dpool.tile([N_CORES, p, width], F32, addr_space="Shared",
                          name=f"arout_{tag}")
        nc.sync.dma_start(cin[:], src_sbuf[:])
        nc.gpsimd.collective_compute(
            "AllGather", AX.bypass,
            replica_groups=[list(range(N_CORES))],
            ins=[cin.opt()], outs=[cout.opt()],
        )
        gath = wpool.tile([p, N_CORES, width], F32, name=f"gath_{tag}")
        nc.sync.dma_start(gath[:], cout[:].rearrange("g p c -> p g c"))
        nc.vector.reduce_sum(out_sbuf[:],
                             gath[:].rearrange("p g c -> p c g"), X_AXIS)

    def rsqrt_newton(y, x, tag):
        """y = 1/sqrt(x) on DVE only (bit-trick seed + 2 Newton steps)."""
        p, w = x.shape[0], x.shape[1]
        xi = x[:].bitcast(I32)
        t1 = wpool.tile([p, w], I32, name=f"rsq_t1_{tag}")
        nc.vector.tensor_scalar(t1[:], xi, 1, None, AX.arith_shift_right)
        yi = y[:].bitcast(I32)
        nc.vector.tensor_scalar(yi, t1[:], -1, 0x5f3759df, AX.mult, AX.add)
        h = wpool.tile([p, w], F32, name=f"rsq_h_{tag}")
        for _ in range(2):
            nc.vector.tensor_tensor(h[:], y[:], y[:], AX.mult)
            nc.vector.tensor_tensor(h[:], x[:], h[:], AX.mult)
            nc.vector.tensor_scalar(h[:], h[:], -0.5, 1.5, AX.mult, AX.add)
            nc.vector.tensor_tensor(y[:], y[:], h[:], AX.mult)

    def bn_coeffs(tot, gt, bt, cnt, w, sc, cc, tag):
        """tot [P, 2w] = [sums | sumsqs] -> scale sc [P, w], bias cc [P, w].
        All DVE (Newton rsqrt) so ScalarE keeps its Exp table loaded."""
        p = tot.shape[0]
        mean = wpool.tile([p, w], F32, name=f"mean_{tag}")
        var = wpool.tile([p, w], F32, name=f"var_{tag}")
        nc.vector.tensor_scalar_mul(mean[:], tot[:, 0:w], 1.0 / cnt)
        nc.vector.tensor_scalar_mul(var[:], tot[:, w:2 * w], 1.0 / cnt)
        m2 = wpool.tile([p, w], F32, name=f"m2_{tag}")
        nc.vector.tensor_tensor(m2[:], mean[:], mean[:], AX.mult)
        nc.vector.tensor_tensor(var[:], var[:], m2[:], AX.subtract)
        nc.vector.tensor_scalar_add(var[:], var[:], EPS)
        rstd = wpool.tile([p, w], F32, name=f"rstd_{tag}")
        rsqrt_newton(rstd, var, tag)
        nc.vector.tensor_tensor(sc[:], gt[:], rstd[:], AX.mult)
        tmp = wpool.tile([p, w], F32, name=f"tmpc_{tag}")
        nc.vector.tensor_tensor(tmp[:], sc[:], mean[:], AX.mult)
        nc.vector.tensor_tensor(cc[:], bt[:], tmp[:], AX.subtract)

    with contextlib.ExitStack() as top:
        wpool = top.enter_context(tc.tile_pool(name="wpool", bufs=1))
        dpool = top.enter_context(tc.tile_pool(name="dpool", bufs=1, space="DRAM"))

        # ---- weights / constants ----
        w1t = []
        for k in range(8):
            t = wpool.tile([128, C], BF16, name=f"w1t{k}")
            nc.scalar.dma_start(t[:], W1T[k * 128:(k + 1) * 128, :])
            w1t.append(t)
        wqt, wvt = [], []
        for srcw, dst, nm in ((WQT, wqt, "wq"), (WVT, wvt, "wv")):
            for k in range(2):
                t = wpool.tile([128, C], F32R, name=f"{nm}{k}")
                nc.scalar.dma_start(t[:], srcw[k * 128:(k + 1) * 128, :])
                dst.append(t)
        wkqt = []
        for h in range(HEADS):
            row = []
            for k in range(2):
                t = wpool.tile([128, 128], F32R, name=f"wkq{h}_{k}")
                nc.scalar.dma_start(t[:], WKQ[h][k * 128:(k + 1) * 128, :])
                row.append(t)
            wkqt.append(row)
        w3t = []
        for k in range(2):
            t = wpool.tile([128, CIN], F32R, name=f"w3t{k}")
            nc.scalar.dma_start(t[:], W3T[k * 128:(k + 1) * 128, :])
            w3t.append(t)
        bvrow = wpool.tile([1, C], F32R, name="bvrow")
        nc.scalar.dma_start(bvrow[:], BVR[:])

        bkqt = wpool.tile([128, HEADS], F32, name="bkqt")
        nc.scalar.dma_start(bkqt[:], BKQ[:].rearrange("h p -> p h"))
        bqt = wpool.tile([DH, HEADS], F32, name="bqt")
        nc.scalar.dma_start(bqt[:], BQd[:].rearrange("h d -> d h"))
        g1t = wpool.tile([128, 2], F32, name="g1t")
        b1t = wpool.tile([128, 2], F32, name="b1t")
        nc.scalar.dma_start(g1t[:], GB1[0].rearrange("(m p) -> p m", p=128))
        nc.scalar.dma_start(b1t[:], GB1[1].rearrange("(m p) -> p m", p=128))
        g2t = wpool.tile([DH, HEADS], F32, name="g2t")
        b2t = wpool.tile([DH, HEADS], F32, name="b2t")
        nc.scalar.dma_start(g2t[:], GB2[0].rearrange("(h d) -> d h", d=DH))
        nc.scalar.dma_start(b2t[:], GB2[1].rearrange("(h d) -> d h", d=DH))
        g3t = wpool.tile([128, 8], F32, name="g3t")
        b3t = wpool.tile([128, 8], F32, name="b3t")
        nc.scalar.dma_start(g3t[:], GB3[0].rearrange("(m p) -> p m", p=128))
        nc.scalar.dma_start(b3t[:], GB3[1].rearrange("(m p) -> p m", p=128))

        # constants: ones pad (bitcast to f32r where needed), warm-chain srcs
        onespad = wpool.tile([128, 128], F32, name="onespad")
        nc.vector.memset(onespad[:], 1.0)
        warmx = wpool.tile([128, 512], F32, name="warmx")
        nc.vector.memset(warmx[:], 0.0)

        def warm_chain(pool, tag, shape, bufs=1, k=WARM_K):
            """Dummy accumulate-matmul chain: keeps the PE clock warm during
            a collective window.  Allocates from the given live PSUM pool."""
            wps = pool.tile(shape, F32, name=f"warm_{tag}", tag=tag, bufs=bufs)
            for i in range(k):
                nc.tensor.matmul(wps[0:128, 0:512], onespad[:].bitcast(F32R),
                                 warmx[:].bitcast(F32R),
                                 start=(i == 0), stop=(i == k - 1))

        # stats accumulators
        S1 = wpool.tile([128, 12], F32, name="S1")   # conv1 sums   (mt*6+cg)
        Q1 = wpool.tile([128, 12], F32, name="Q1")   # conv1 sumsqs
        S2 = wpool.tile([DH, 12], F32, name="S2")    # attn sums    (h*3+gi)
        Q2 = wpool.tile([DH, 12], F32, name="Q2")
        S3 = wpool.tile([128, 24], F32, name="S3")   # W3 sums      (mt*3+ci)
        Q3 = wpool.tile([128, 24], F32, name="Q3")

        # resident bf16 X tiles covering cols 0:1372 (conv input + residual)
        XAB = [wpool.tile([128, 1024], BF16, name=f"xab{k}") for k in range(8)]
        XC = [wpool.tile([128, 348], BF16, name=f"xc{k}") for k in range(8)]
        # residual slices per finale chunk cg in 0..2
        XRES = [[XAB[k][:, 0:512] for k in range(8)],
                [XAB[k][:, 512:1024] for k in range(8)],
                [XC[k][:] for k in range(8)]]
        OUT2 = [wpool.tile([128, NQ], F32R, name=f"out2_{m}") for m in range(2)]

        with contextlib.ExitStack() as ph_a:
            qpool = ph_a.enter_context(tc.tile_pool(name="qpool", bufs=1))
            KHAT = [qpool.tile([128, N], F32R, name=f"khat{h}") for h in range(HEADS)]
            QHAT = [qpool.tile([128, NQ], F32R, name=f"qhat{h}") for h in range(HEADS)]
            # per-head blocks of 22*65 cols: [v^T (64) | ones] per token tile
            VTON = qpool.tile([128, HEADS * 22 * 65], F32R, name="vton")
            # bf16 shadow for the tail-query AV matmuls (Schraudolph path)
            VTONB = qpool.tile([128, HEADS * 22 * 65], BF16, name="vtonb")
            # ones columns (softmax denominator rows), one strided copy per head
            for vt in (VTON, VTONB):
                for h in range(HEADS):
                    dst = vt[:, h * 1430:(h + 1) * 1430].rearrange(
                        "p (t c) -> p t c", t=22)[:, :, DH:DH + 1]
                    src = onespad[:, 0:22].rearrange("p (t c) -> p t c", t=22)
                    nc.vector.tensor_copy(dst, src)
            for h in range(HEADS):
                nc.scalar.dma_start(QHAT[h][DH:128, :], RELd[h])

            with contextlib.ExitStack() as ph1:
                y1pool = ph1.enter_context(tc.tile_pool(name="y1pool", bufs=1))
                Y1 = [y1pool.tile([128, N], F32R, name=f"y1_{m}") for m in range(2)]
                sq1pool = ph1.enter_context(tc.tile_pool(name="sq1pool", bufs=2))

                # ---- phase 1: conv1 (y1 = W1 @ x) bf16, stats on ScalarE ----
                with tc.tile_pool(name="xbpool", bufs=1) as xbpool, \
                     tc.tile_pool(name="psum1", bufs=4, space="PSUM") as psum1:
                    # warm the PE clock while weights/X stream in
                    warm_chain(psum1, "warm0", [128, 512], k=16)
                    dma_engs = (nc.sync, nc.gpsimd)
                    cstat = 0
                    for g, (gbase, gw, subs) in enumerate(XGRP):
                        if g == 0:
                            xts = XAB
                        elif g == 1:
                            xts = XC
                        else:
                            pfx = "xb" if g == 2 else "xd"
                            xts = [xbpool.tile([128, gw], BF16,
                                               name=f"{pfx}{g}_{k}",
                                               tag=f"{pfx}{k}")
                                   for k in range(8)]
                        for k in range(8):
                            xo = XOFF[(g, k)]
                            eng = dma_engs[0] if g < 2 else dma_engs[1]
                            eng.dma_start(
                                xts[k][:], Xd[xo:xo + 128 * gw].rearrange(
                                    "(p f) -> p f", p=128))
                        for so, ssz in subs:
                            off = gbase + so
                            for mt in range(2):
                                ps = psum1.tile([128, ssz], F32, name="pconv",
                                                tag="pconv",
                                                padded_shape=[128, 512])
                                for k in range(8):
                                    nc.tensor.matmul(
                                        ps[:], w1t[k][:, mt * 128:(mt + 1) * 128],
                                        xts[k][:, so:so + ssz],
                                        start=(k == 0), stop=(k == 7))
                                idx = mt * 6 + cstat
                                nc.scalar.activation(Y1[mt][:, off:off + ssz],
                                                     ps[:], AF.Copy,
                                                     accum_out=S1[:, idx:idx + 1])
                                sq = sq1pool.tile([128, ssz], F32, name="sqs",
                                                  tag="sqs",
                                                  padded_shape=[128, 512])
                                nc.scalar.activation(sq[:], ps[:], AF.Square,
                                                     accum_out=Q1[:, idx:idx + 1])
                            cstat += 1

                # ---- phase 1b: BN1 AllReduce + coeffs (DVE); PE keeps warm ----
                s1sum = wpool.tile([128, 2], F32, name="s1sum")
                q1sum = wpool.tile([128, 2], F32, name="q1sum")
                nc.vector.reduce_sum(s1sum[:], S1[:].rearrange("p (m c) -> p m c", c=6), X_AXIS)
                nc.vector.reduce_sum(q1sum[:], Q1[:].rearrange("p (m c) -> p m c", c=6), X_AXIS)
                st1 = wpool.tile([128, 4], F32, name="st1")
                nc.vector.tensor_copy(st1[:, 0:2], s1sum[:])
                nc.vector.tensor_copy(st1[:, 2:4], q1sum[:])
                tot1 = wpool.tile([128, 4], F32, name="tot1")
                stats_allreduce(st1, 4, tot1, "bn1")
                s1c = wpool.tile([128, 2], F32, name="s1c")
                c1c = wpool.tile([128, 2], F32, name="c1c")
                bn_coeffs(tot1, g1t, b1t, CNT12, 2, s1c, c1c, "bn1")

                # ---- phase 2: out1 = relu(s*y1+c) in place, 3-engine split ----
                OUT1 = [Y1[m][:] for m in range(2)]
                for mt in range(2):
                    for cg, (off, sz) in enumerate(CGX):
                        ap = Y1[mt][:, off:off + sz]
                        if cg < 4:
                            nc.scalar.activation(ap, ap, AF.Relu,
                                                 bias=c1c[:, mt:mt + 1],
                                                 scale=s1c[:, mt:mt + 1])
                        else:
                            nc.vector.tensor_scalar(ap, ap,
                                                    s1c[:, mt:mt + 1],
                                                    c1c[:, mt:mt + 1],
                                                    AX.mult, AX.add)
                            nc.vector.tensor_scalar(ap, ap, 0.0, None, AX.max)

                # ---- phase 3a: vT = out1^T @ WvT (+bv), into VTON (ScalarE) ----
                with tc.tile_pool(name="psum3", bufs=1, space="PSUM") as psum3:
                    # keep the PE clock warm through the BN1 collective window
                    warm_chain(psum3, "warm1", [128, 512], k=70)
                    for t, (mo, msz) in enumerate(MT22):
                        ps = psum3.tile([128, C], F32, name="pvt", tag="pvt", bufs=2)
                        nc.tensor.matmul(ps[0:msz, :],
                                         onespad[0:1, 0:msz].bitcast(F32R),
                                         bvrow[:], start=True, stop=False)
                        for k in range(2):
                            nc.tensor.matmul(ps[0:msz, :], OUT1[k][:, mo:mo + msz],
                                             wvt[k][:], start=False, stop=(k == 1))
                        # one strided copy for all 4 heads: [msz, 4, 64]
                        dst = VTON[0:msz, :].rearrange(
                            "p (h c) -> p h c", h=HEADS)[:, :, t * 65:t * 65 + DH]
                        src = ps[0:msz, :].rearrange("p (h c) -> p h c", h=HEADS)
                        nc.scalar.activation(dst, src, AF.Copy)
                        dstb = VTONB[0:msz, :].rearrange(
                            "p (h c) -> p h c", h=HEADS)[:, :, t * 65:t * 65 + DH]
                        nc.vector.tensor_copy(dstb, src)

                    # ---- phase 3b: KHAT = [k;q] packed, QHAT q-half ----
                    # bias adds split Scalar (Identity+bias) / DVE; GpSimd
                    # cannot read PSUM.
                    for h in range(HEADS):
                        hs = h * DH
                        for ci, (off, sz) in enumerate(CH6):
                            ps = psum3.tile([128, sz], F32, name="pkh", tag="pkh",
                                            bufs=3)
                            for k in range(2):
                                nc.tensor.matmul(ps[:], wkqt[h][k][:],
                                                 OUT1[k][:, off:off + sz],
                                                 start=(k == 0), stop=(k == 1))
                            if ci % 2 == 0:
                                nc.scalar.activation(KHAT[h][:, off:off + sz],
                                                     ps[:], AF.Identity,
                                                     bias=bkqt[:, h:h + 1])
                            else:
                                nc.vector.tensor_scalar(KHAT[h][:, off:off + sz],
                                                        ps[:], bkqt[:, h:h + 1],
                                                        None, AX.add)
                        for (off, sz) in CH3:
                            pq = psum3.tile([DH, sz], F32, name="pqh", tag="pqh",
                                            bufs=2)
                            for k in range(2):
                                nc.tensor.matmul(pq[:], wqt[k][:, hs:hs + DH],
                                                 OUT1[k][:, off:off + sz],
                                                 start=(k == 0), stop=(k == 1))
                            nc.vector.tensor_scalar(QHAT[h][0:DH, off:off + sz], pq[:],
                                                    bqt[:, h:h + 1], None, AX.add)

            # ---- phase 4: attention ----
            # queries 0:1024 per head run as a Scalar-dense 1024-wide loop;
            # the 348-query tails of two heads are interleaved afterwards so
            # their serial logits->exp->AV chains overlap.  The softmax
            # denominator reciprocal is exp(-ln(d)) on ScalarE (a [1,512]
            # DVE reciprocal costs 3us; ScalarE does the pair in 1.1us).
            with tc.tile_pool(name="oattp", bufs=1) as oattp, \
                 tc.tile_pool(name="epool", bufs=1) as epool:
                OATT = [oattp.tile([DH, NQ], F32R, name=f"oatt{h}")
                        for h in range(HEADS)]

                def post_group(pool, pav, h, off, sz, idx, pbtag, pbbufs):
                    rcr = epool.tile([1, sz], F32, name="rcr", tag="rcr",
                                     bufs=2, padded_shape=[1, 512])
                    nc.vector.reciprocal(rcr[:], pav[DH:65, :])
                    pb = pool.tile([DH, sz], F32, name="pb", tag=pbtag,
                                   bufs=pbbufs, padded_shape=[DH, 512])
                    nc.tensor.matmul(pb[:], onespad[0:1, 0:DH], rcr[:],
                                     start=True, stop=True)
                    pbs = epool.tile([DH, sz], F32, name="pbs", tag="pbs",
                                     bufs=2, padded_shape=[DH, 512])
                    nc.vector.tensor_copy(pbs[:], pb[:])
                    nc.vector.scalar_tensor_tensor(
                        OATT[h][:, off:off + sz], pav[0:DH, :], 1.0, pbs[:],
                        AX.mult, AX.mult, accum_out=S2[:, idx:idx + 1])
                    sq2 = epool.tile([DH, sz], F32, name="sq2", tag="sq2",
                                     bufs=1, padded_shape=[DH, 512])
                    nc.vector.scalar_tensor_tensor(
                        sq2[:], OATT[h][:, off:off + sz], 1.0,
                        OATT[h][:, off:off + sz],
                        AX.mult, AX.mult, accum_out=Q2[:, idx:idx + 1])

                with tc.tile_pool(name="psum4a", bufs=1, space="PSUM") as ps4a:
                    for h in range(HEADS):
                        pavs = [ps4a.tile([65, 512], F32, name=f"pav{si}",
                                          tag=f"pav{si}", bufs=1)
                                for si in range(2)]
                        for t, (mo, msz) in enumerate(MT22):
                            ps = ps4a.tile([128, 1024], F32, name="ps", tag="ps",
                                           bufs=2)
                            for so in (0, 512):
                                nc.tensor.matmul(ps[0:msz, so:so + 512],
                                                 KHAT[h][:, mo:mo + msz],
                                                 QHAT[h][:, so:so + 512],
                                                 start=True, stop=True)
                            # exp split: ScalarE real exp on cols 0:512, DVE
                            # Schraudolph-bf16 on cols 512:1024 (the Scalar
                            # 1024-wide exp was the g1 iteration bottleneck)
                            e = epool.tile([128, 512], F32R, name="e", tag="e",
                                           bufs=3)
                            nc.scalar.activation(e[0:msz, :], ps[0:msz, 0:512],
                                                 AF.Exp)
                            eb = epool.tile([128, 512], BF16, name="eb",
                                            tag="eb", bufs=3)
                            nc.vector.tensor_scalar(eb[0:msz, :].bitcast(I16),
                                                    ps[0:msz, 512:1024],
                                                    SCHR_A16, SCHR_B16,
                                                    AX.mult, AX.add)
                            base = (h * 22 + t) * 65
                            nc.tensor.matmul(pavs[0][:],
                                             VTON[0:msz, base:base + 65],
                                             e[0:msz, :],
                                             start=(t == 0), stop=(t == 21))
                            nc.tensor.matmul(pavs[1][:],
                                             VTONB[0:msz, base:base + 65],
                                             eb[0:msz, :],
                                             start=(t == 0), stop=(t == 21))
                        for si in range(2):
                            post_group(ps4a, pavs[si], h, si * 512, 512,
                                       h * 3 + si, "pb", 1)

                with tc.tile_pool(name="psum4b", bufs=1, space="PSUM") as ps4b:
                    for pair in ((0, 1), (2, 3)):
                        pav2 = {h: ps4b.tile([65, 348], F32, name=f"pav2_{h}",
                                             tag=f"pav2_{h % 2}", bufs=1,
                                             padded_shape=[65, 512])
                                for h in pair}
                        for t, (mo, msz) in enumerate(MT22):
                            for h in pair:
                                ps = ps4b.tile([128, 348], F32, name="ps2",
                                               tag="ps2", bufs=4,
                                               padded_shape=[128, 512])
                                nc.tensor.matmul(ps[0:msz, :],
                                                 KHAT[h][:, mo:mo + msz],
                                                 QHAT[h][:, 1024:1372],
                                                 start=True, stop=True)
                                base = (h * 22 + t) * 65
                                if h % 2 == 0:
                                    e2 = epool.tile([128, 348], F32R, name="e2f",
                                                    tag="e2f", bufs=2,
                                                    padded_shape=[128, 512])
                                    nc.scalar.activation(e2[0:msz, :],
                                                         ps[0:msz, :], AF.Exp)
                                    vt = VTON
                                else:
                                    e2 = epool.tile([128, 348], BF16, name="e2b",
                                                    tag="e2b", bufs=2,
                                                    padded_shape=[128, 512])
                                    nc.vector.tensor_scalar(
                                        e2[0:msz, :].bitcast(I16), ps[0:msz, :],
                                        SCHR_A16, SCHR_B16, AX.mult, AX.add)
                                    vt = VTONB
                                nc.tensor.matmul(pav2[h][:],
                                                 vt[0:msz, base:base + 65],
                                                 e2[0:msz, :],
                                                 start=(t == 0), stop=(t == 21))
                        for h in pair:
                            post_group(ps4b, pav2[h], h, 1024, 348,
                                       h * 3 + 2, "pb2", 2)
                            # move raw head output into OUT2 layout
                            nc.sync.dma_start(
                                OUT2[h // 2][(h % 2) * DH:(h % 2) * DH + DH, :],
                                OATT[h][:])

                # ---- phase 5: BN2 AllReduce + fused Relu apply ----
                s2sum = wpool.tile([DH, 4], F32, name="s2sum")
                q2sum = wpool.tile([DH, 4], F32, name="q2sum")
                nc.vector.reduce_sum(s2sum[:], S2[:].rearrange("p (h c) -> p h c", c=3), X_AXIS)
                nc.vector.reduce_sum(q2sum[:], Q2[:].rearrange("p (h c) -> p h c", c=3), X_AXIS)
                st2 = wpool.tile([DH, 8], F32, name="st2")
                nc.vector.tensor_copy(st2[:, 0:4], s2sum[:])
                nc.vector.tensor_copy(st2[:, 4:8], q2sum[:])
                tot2 = wpool.tile([DH, 8], F32, name="tot2")
                stats_allreduce(st2, 8, tot2, "bn2")
                s2c = wpool.tile([DH, 4], F32, name="s2c")
                c2c = wpool.tile([DH, 4], F32, name="c2c")
                bn_coeffs(tot2, g2t, b2t, CNT2, 4, s2c, c2c, "bn2")
                # rearrange [64,4] head coeffs -> [128,2] OUT2 channel layout
                s2c128 = wpool.tile([128, 2], F32, name="s2c128")
                c2c128 = wpool.tile([128, 2], F32, name="c2c128")
                for s in range(2):
                    nc.sync.dma_start(
                        s2c128[s * DH:(s + 1) * DH, :],
                        s2c[:].rearrange("d (m s) -> d m s", s=2)[:, :, s])
                    nc.sync.dma_start(
                        c2c128[s * DH:(s + 1) * DH, :],
                        c2c[:].rearrange("d (m s) -> d m s", s=2)[:, :, s])
                for m in range(2):
                    nc.scalar.activation(OUT2[m][:], OUT2[m][:], AF.Relu,
                                         bias=c2c128[:, m:m + 1],
                                         scale=s2c128[:, m:m + 1])

        # ---- phase 6: y3 = W3 @ out2; stats Scalar+DVE; finale 3-engine ----
        with tc.tile_pool(name="y3pool", bufs=1) as y3pool, \
             tc.tile_pool(name="fpool", bufs=2) as fpool, \
             tc.tile_pool(name="psum6", bufs=3, space="PSUM") as psum6:
            # keep the PE clock warm through the BN2 collective window
            warm_chain(psum6, "warm2", [128, 512], bufs=1)
            Y3 = [y3pool.tile([128, NQ], BF16, name=f"y3_{m}") for m in range(8)]
            for mt in range(8):
                for ci, (off, sz) in enumerate(CH3):
                    ps = psum6.tile([128, sz], F32, name="pw3", tag="pw3")
                    for k in range(2):
                        nc.tensor.matmul(ps[:], w3t[k][:, mt * 128:(mt + 1) * 128],
                                         OUT2[k][:, off:off + sz],
                                         start=(k == 0), stop=(k == 1))
                    idx = mt * 3 + ci
                    nc.scalar.activation(Y3[mt][:, off:off + sz], ps[:], AF.Copy,
                                         accum_out=S3[:, idx:idx + 1])
                    sq = fpool.tile([128, sz], F32, name="sq3", tag="sq3")
                    nc.vector.scalar_tensor_tensor(
                        sq[:], Y3[mt][:, off:off + sz], 1.0,
                        Y3[mt][:, off:off + sz], AX.mult, AX.mult,
                        accum_out=Q3[:, idx:idx + 1])

            s3sum = wpool.tile([128, 8], F32, name="s3sum")
            q3sum = wpool.tile([128, 8], F32, name="q3sum")
            nc.vector.reduce_sum(s3sum[:], S3[:].rearrange("p (m c) -> p m c", c=3), X_AXIS)
            nc.vector.reduce_sum(q3sum[:], Q3[:].rearrange("p (m c) -> p m c", c=3), X_AXIS)
            st3 = wpool.tile([128, 16], F32, name="st3")
            nc.vector.tensor_copy(st3[:, 0:8], s3sum[:])
            nc.vector.tensor_copy(st3[:, 8:16], q3sum[:])
            tot3 = wpool.tile([128, 16], F32, name="tot3")
            stats_allreduce(st3, 16, tot3, "bn3")
            s3c = wpool.tile([128, 8], F32, name="s3c")
            c3c = wpool.tile([128, 8], F32, name="c3c")
            bn_coeffs(tot3, g3t, b3t, CNT2, 8, s3c, c3c, "bn3")

            # finale: out = relu(s3*y3 + c3 + x); op1 on DVE (stt with the
            # resident bf16 X tiles), op2 split Scalar (cg 0,2) / DVE (cg 1)
            for mt in range(8):
                for cg in range(3):
                    off, sz = CGX[cg]
                    tf = fpool.tile([128, sz], BF16, name="tf", tag="tf", bufs=3,
                                    padded_shape=[128, 512])
                    nc.vector.scalar_tensor_tensor(
                        tf[:], Y3[mt][:, off:off + sz], s3c[:, mt:mt + 1],
                        XRES[cg][mt], AX.mult, AX.add)
                    to = fpool.tile([128, sz], F32, name="to", tag="to", bufs=3,
                                    padded_shape=[128, 512])
                    nc.scalar.activation(to[:], tf[:], AF.Relu,
                                         bias=c3c[:, mt:mt + 1])
                    oo = OOFF[(mt, cg)]
                    nc.sync.dma_start(OUTd[oo:oo + 128 * sz].rearrange(
                        "(p f) -> p f", p=128), to[:])


_NC_CACHE = {}


def _get_program():
    if "nc" not in _NC_CACHE:
        _NC_CACHE["nc"] = build_program()
    return _NC_CACHE["nc"]


def _host_prep(inputs):
    x = np.ascontiguousarray(inputs["x"].reshape(B, CIN, N))
    rel = (inputs["rel_h"] + inputs["rel_w"] + inputs["rel_d"]).reshape(HEADS, DH, N)
    rel = np.ascontiguousarray(rel.astype(np.float32))
    W1T = np.ascontiguousarray(inputs["W1"].T.astype(ml_dtypes.bfloat16))
    WQT = np.ascontiguousarray(inputs["Wq"].T.astype(np.float32))
    WKT = np.ascontiguousarray(inputs["Wk"].T.astype(np.float32))
    WVT = np.ascontiguousarray(inputs["Wv"].T.astype(np.float32))
    W3T = np.ascontiguousarray(inputs["W3"].T.astype(np.float32))
    WKQ = np.stack([np.concatenate([WKT[:, h * DH:(h + 1) * DH],
                                    WQT[:, h * DH:(h + 1) * DH]], axis=1)
                    for h in range(HEADS)]).astype(np.float32)
    bq, bk, bv = inputs["bq"], inputs["bk"], inputs["bv"]
    BKQ = np.stack([np.concatenate([bk[h * DH:(h + 1) * DH], bq[h * DH:(h + 1) * DH]])
                    for h in range(HEADS)]).astype(np.float32)
    BQ = bq.reshape(HEADS, DH).astype(np.float32)
    BVR = bv.reshape(1, C).astype(np.float32)
    GB1 = np.stack([inputs["g1"], inputs["b1"]]).astype(np.float32)
    GB2 = np.stack([inputs["g2"], inputs["b2"]]).astype(np.float32)
    GB3 = np.stack([inputs["g3"], inputs["b3"]]).astype(np.float32)

    in_maps = []
    for c in range(N_CORES):
        b, s = c // 2, c % 2
        xb = np.roll(x[b], -s * NQ, axis=1).astype(ml_dtypes.bfloat16)
        # tiled layout: contiguous [128, sz] blocks (1 DMA descriptor each)
        xt = np.empty(CIN * N, ml_dtypes.bfloat16)
        for g, (gbase, gw, _subs) in enumerate(XGRP):
            for k in range(8):
                o = XOFF[(g, k)]
                xt[o:o + 128 * gw] = xb[k * 128:(k + 1) * 128,
                                        gbase:gbase + gw].reshape(-1)
        relc = np.ascontiguousarray(rel[:, :, s * NQ:(s + 1) * NQ])
        in_maps.append({
            "X": xt, "W1T": W1T, "WQT": WQT,
            "WVT": WVT, "W3T": W3T, "WKQ": WKQ, "REL": relc, "BKQ": BKQ,
            "BQ": BQ, "BVR": BVR, "GB1": GB1, "GB2": GB2, "GB3": GB3,
        })
    return in_maps


def run(inputs, trace=False, trace_kwargs=None):
    from concourse import bass_utils
    nc = _get_program()
    in_maps = _host_prep(inputs)
    res = bass_utils.run_bass_kernel_spmd(
        nc, in_maps, core_ids=list(range(N_CORES)), trace=trace,
        **(trace_kwargs or {}))
    out = np.empty((B, CIN, N), np.float32)
    for c in range(N_CORES):
        b, s = c // 2, c % 2
        flat = np.asarray(res.results[c]["OUT"]).reshape(-1)
        oc = np.empty((CIN, NQ), np.float32)
        for mt in range(8):
            for cg in range(3):
                off, sz = CGX[cg]
                o = OOFF[(mt, cg)]
                oc[mt * 128:(mt + 1) * 128, off:off + sz] = \
                    flat[o:o + 128 * sz].reshape(128, sz)
        out[b, :, s * NQ:(s + 1) * NQ] = oc
    return out.reshape(B, CIN, 14, 14, 14), res


def kernel(**inputs):
    out, _ = run(inputs, trace=False)
    return out
